# revision 20
# baseline (speedup 1.0000x reference)
"""Bass/Trainium2 kernel for nn_AttentionOU (sparse policy attention).

Contract: kernel(**inputs) takes FULL inputs (B=64), shards batch across 8
NeuronCores (8 per core), runs one SPMD Bass program, gathers full output.

Per-core program layout (all feature-major / key-major so matmuls compose
without runtime transposes of big intermediates):
  - x loaded row-major (contiguous DMA), cast to bf16, transposed on PE
    (128x128 tiles) into x^T bf16; rows 64..384 also transposed in fp32 for
    the divide-MLP (which must run fp32: bf16 flips ~32/16384 argmax
    decisions -> large output error).
  - q^T,k^T = Wq/Wk-stationary matmuls -> feature-major bf16.
  - V       = x^T-stationary matmuls -> key-major bf16 [128, 12, 65] tiles:
              per-head 64 V columns + a ones column (so the PV matmul also
              emits the softmax denominator for free).
  - S^T     = k^T.T @ q^T (key-major). No max-subtraction: |logits| <= ~3,
              exp is safe in fp32; eps-term difference is ~1e-8 absolute.
  - softmax: exp on ACT; the policy mask is two cheap row/col multiplies
              (mask only kills template<->search-class1 pairs); reference's
              (a + eps/Nq)/(sum + eps) reproduced exactly via a pseudo-key:
              a_pseudo = eps/Nq, V_pseudo = sum_j V_j, ones-slot = Nq.
  - O^T     = V.T @ a^T -> feature-major, feeds proj as stationary operand.
  - proj    = O^T-stationary -> y ROW-major -> contiguous stores.
"""

import numpy as np
from contextlib import ExitStack

import concourse.bass as bass
import concourse.tile as tile
from concourse import bacc, mybir
from concourse.masks import make_identity

F32 = mybir.dt.float32
BF16 = mybir.dt.bfloat16
AF = mybir.ActivationFunctionType
ALU = mybir.AluOpType
AX = mybir.AxisListType

BPC = 8          # batch per core
NCORES = 8
NV = 385         # keys
NQ = 321         # queries
C = 768
H = 12
HD = 64
S = 256
TPL = 64
EPS = 1e-6
SCALE = HD ** -0.5


def _bcast(t_ap, parts):
    """Broadcast a [1, N] AP across `parts` partitions (0-stride)."""
    return bass.AP(
        tensor=t_ap.tensor,
        offset=t_ap.offset,
        ap=[[0, parts]] + [list(d) for d in t_ap.ap[1:]],
    )


def _row_ap(dram_handle, off, n):
    return bass.AP(tensor=dram_handle[:].tensor, offset=off, ap=[[0, 1], [1, n]])


def build(nc):
    x_d = nc.dram_tensor("x", [BPC, NV, C], F32, kind="ExternalInput")
    qkvw_d = nc.dram_tensor("qkv_w", [3 * C, C], F32, kind="ExternalInput")
    qkvb_d = nc.dram_tensor("qkv_b", [3 * C], F32, kind="ExternalInput")
    projw_d = nc.dram_tensor("proj_w", [C, C], F32, kind="ExternalInput")
    projb_d = nc.dram_tensor("proj_b", [C], F32, kind="ExternalInput")
    dp1w_d = nc.dram_tensor("dp1_w", [384, 2 * C], F32, kind="ExternalInput")
    dp1b_d = nc.dram_tensor("dp1_b", [384], F32, kind="ExternalInput")
    dp2w_d = nc.dram_tensor("dp2_w", [192, 384], F32, kind="ExternalInput")
    dp2b_d = nc.dram_tensor("dp2_b", [192], F32, kind="ExternalInput")
    dp3w_d = nc.dram_tensor("dp3_w", [2, 192], F32, kind="ExternalInput")
    dp3b_d = nc.dram_tensor("dp3_b", [2], F32, kind="ExternalInput")
    out_d = nc.dram_tensor("out", [BPC, NQ, C], F32, kind="ExternalOutput")

    with tile.TileContext(nc) as tc, ExitStack() as ctx:
        wp = ctx.enter_context(tc.tile_pool(name="w", bufs=1))
        bp = ctx.enter_context(tc.tile_pool(name="bb", bufs=2))
        ap_ = ctx.enter_context(tc.tile_pool(name="aa", bufs=3))
        ps_mm = ctx.enter_context(tc.tile_pool(name="psmm", bufs=2, space="PSUM"))
        ps_att = ctx.enter_context(tc.tile_pool(name="psatt", bufs=2, space="PSUM"))
        ps_sm = ctx.enter_context(tc.tile_pool(name="pssm", bufs=2, space="PSUM"))

        identf = wp.tile([128, 128], F32, tag="identf")
        make_identity(nc, identf)
        identb = wp.tile([128, 128], BF16, tag="identb")
        nc.vector.tensor_copy(out=identb, in_=identf)
        ones_col = wp.tile([128, 1], BF16, tag="ones")
        nc.vector.memset(ones_col, 1.0)
        ones_row_f = wp.tile([1, 128], F32, tag="onesrf")
        nc.vector.memset(ones_row_f, 1.0)

        # ================= weights (once) =================
        stage_ctx = ExitStack()
        stage = stage_ctx.enter_context(tc.tile_pool(name="stage", bufs=1))
        # qkv_w -> bf16 row tiles -> transposed [infeat, 3C]
        wqkvT = [wp.tile([128, 3 * C], BF16, tag=f"wqkv{c}", name=f"wqkvT{c}") for c in range(6)]
        for r in range(18):
            t = stage.tile([128, C], BF16, tag="qkvrm", name=f"qkvrm{r}", bufs=2)
            nc.gpsimd.dma_start(out=t, in_=qkvw_d[r * 128:(r + 1) * 128, :])
            for c in range(6):
                pst = ps_mm.tile([128, 128], BF16, tag="mm", name="pst")
                nc.tensor.transpose(pst, t[:, c * 128:(c + 1) * 128], identb)
                nc.vector.tensor_copy(out=wqkvT[c][:, r * 128:(r + 1) * 128], in_=pst)
        wpT = [wp.tile([128, C], BF16, tag=f"wp{c}", name=f"wpT{c}") for c in range(6)]
        for r in range(6):
            t = stage.tile([128, C], BF16, tag="pjrm", name=f"pjrm{r}", bufs=2)
            nc.gpsimd.dma_start(out=t, in_=projw_d[r * 128:(r + 1) * 128, :])
            for c in range(6):
                pst = ps_mm.tile([128, 128], BF16, tag="mm", name="pst")
                nc.tensor.transpose(pst, t[:, c * 128:(c + 1) * 128], identb)
                nc.vector.tensor_copy(out=wpT[c][:, r * 128:(r + 1) * 128], in_=pst)
        # dp1_w fp32 [384, 1536] -> dp1aT/dp1bT [768, 384] chunks
        dp1aT = [wp.tile([128, 384], F32, tag=f"d1a{c}", name=f"d1aT{c}") for c in range(6)]
        dp1bT = [wp.tile([128, 384], F32, tag=f"d1b{c}", name=f"d1bT{c}") for c in range(6)]
        for r in range(3):
            t = stage.tile([128, 2 * C], F32, tag="d1rm", name=f"d1rm{r}", bufs=1)
            nc.sync.dma_start(out=t, in_=dp1w_d[r * 128:(r + 1) * 128, :])
            for c in range(6):
                for (dstl, base) in ((dp1aT, 0), (dp1bT, C)):
                    pst = ps_mm.tile([128, 128], F32, tag="mm", name="pst")
                    nc.tensor.transpose(pst, t[:, base + c * 128: base + (c + 1) * 128], identf)
                    nc.vector.tensor_copy(out=dstl[c][:, r * 128:(r + 1) * 128], in_=pst)
        dp2_rm0 = stage.tile([128, 384], F32, tag="d2rm0")
        nc.sync.dma_start(out=dp2_rm0, in_=dp2w_d[0:128, :])
        dp2_rm1 = stage.tile([64, 384], F32, tag="d2rm1")
        nc.sync.dma_start(out=dp2_rm1, in_=dp2w_d[128:192, :])
        dp2T = []
        for c in range(3):
            t = wp.tile([128, 192], F32, tag=f"d2{c}")
            pst = ps_mm.tile([128, 192], F32, tag="mm", name="pst")
            nc.tensor.transpose(pst[:, 0:128], dp2_rm0[:, c * 128:(c + 1) * 128], identf)
            nc.tensor.transpose(pst[:, 128:192], dp2_rm1[:, c * 128:(c + 1) * 128], identf[0:64, 0:64])
            nc.vector.tensor_copy(out=t, in_=pst)
            dp2T.append(t)
        dp3_rm = stage.tile([2, 192], F32, tag="d3rm")
        nc.sync.dma_start(out=dp3_rm, in_=dp3w_d[:, :])
        dp3T0 = wp.tile([128, 2], F32, tag="d3t0")
        pst = ps_sm.tile([128, 2], F32, tag="sm", name="pst3")
        nc.tensor.transpose(pst, dp3_rm[:, 0:128], identf[0:2, 0:2])
        nc.vector.tensor_copy(out=dp3T0, in_=pst)
        dp3T1 = wp.tile([64, 2], F32, tag="d3t1")
        pst = ps_sm.tile([64, 2], F32, tag="sm", name="pst4")
        nc.tensor.transpose(pst, dp3_rm[:, 128:192], identf[0:2, 0:2])
        nc.vector.tensor_copy(out=dp3T1, in_=pst)
        w3d0 = wp.tile([128, 1], F32, tag="w3d0")
        nc.vector.tensor_tensor(out=w3d0, in0=dp3T0[:, 1:2], in1=dp3T0[:, 0:1], op=ALU.subtract)
        w3d1 = wp.tile([64, 1], F32, tag="w3d1")
        nc.vector.tensor_tensor(out=w3d1, in0=dp3T1[:, 1:2], in1=dp3T1[:, 0:1], op=ALU.subtract)

        stage_ctx.close()

        # bias rows (contiguous) + transposed columns where needed
        qkvb_row = wp.tile([1, 3 * C], F32, tag="qkvbr")
        nc.sync.dma_start(out=qkvb_row, in_=_row_ap(qkvb_d, 0, 3 * C))
        qkvb_col = wp.tile([128, 12], F32, tag="qkvbc")
        for m in range(12):
            pst = ps_sm.tile([128, 1], F32, tag="sm", name="pstb")
            nc.tensor.transpose(pst, qkvb_row[0:1, m * 128:(m + 1) * 128], identf[0:1, 0:1])
            nc.vector.tensor_copy(out=qkvb_col[:, m:m + 1], in_=pst)
        qb_s = wp.tile([128, 6], F32, tag="qbs")
        nc.vector.tensor_scalar_mul(qb_s, qkvb_col[:, 0:6], SCALE)
        vb_rep = wp.tile([128, C], F32, tag="vbrep")
        nc.gpsimd.dma_start(out=vb_rep, in_=bass.AP(tensor=qkvb_d[:].tensor, offset=2 * C,
                                                    ap=[[0, 128], [1, C]]))
        projb_rep = wp.tile([128, C], F32, tag="pjbrep")
        nc.gpsimd.dma_start(out=projb_rep, in_=bass.AP(tensor=projb_d[:].tensor, offset=0,
                                                       ap=[[0, 128], [1, C]]))
        dp1b_row = wp.tile([1, 384], F32, tag="d1br")
        nc.sync.dma_start(out=dp1b_row, in_=_row_ap(dp1b_d, 0, 384))
        dp1b_col = wp.tile([128, 3], F32, tag="d1bc")
        for m in range(3):
            pst = ps_sm.tile([128, 1], F32, tag="sm", name="pstc")
            nc.tensor.transpose(pst, dp1b_row[0:1, m * 128:(m + 1) * 128], identf[0:1, 0:1])
            nc.vector.tensor_copy(out=dp1b_col[:, m:m + 1], in_=pst)
        dp2b_row = wp.tile([1, 192], F32, tag="d2br")
        nc.sync.dma_start(out=dp2b_row, in_=_row_ap(dp2b_d, 0, 192))
        dp2b_c0 = wp.tile([128, 1], F32, tag="d2b0")
        pst = ps_sm.tile([128, 1], F32, tag="sm", name="pstd")
        nc.tensor.transpose(pst, dp2b_row[0:1, 0:128], identf[0:1, 0:1])
        nc.vector.tensor_copy(out=dp2b_c0, in_=pst)
        dp2b_c1 = wp.tile([64, 1], F32, tag="d2b1")
        pst = ps_sm.tile([64, 1], F32, tag="sm", name="pste")
        nc.tensor.transpose(pst, dp2b_row[0:1, 128:192], identf[0:1, 0:1])
        nc.vector.tensor_copy(out=dp2b_c1, in_=pst)
        dp3b_row = wp.tile([1, 2], F32, tag="d3br")
        nc.sync.dma_start(out=dp3b_row, in_=_row_ap(dp3b_d, 0, 2))
        b3diff = wp.tile([1, 1], F32, tag="b3d")
        nc.vector.tensor_tensor(out=b3diff, in0=dp3b_row[0:1, 1:2], in1=dp3b_row[0:1, 0:1], op=ALU.subtract)

        # ================= per batch item =================
        for b in range(BPC):
            # --- load x row-major, cast, transpose ---
            xr = []
            for rc in range(4):
                rows = 128 if rc < 3 else 1
                t = bp.tile([rows, C], F32, tag=f"xr{rc}", name=f"xr{rc}", bufs=1)
                nc.sync.dma_start(out=t, in_=x_d[b, rc * 128:rc * 128 + rows, :])
                xr.append(t)
            xrb = []
            for rc in range(4):
                rows = 128 if rc < 3 else 1
                t = bp.tile([rows, C], BF16, tag=f"xrb{rc}", name=f"xrb{rc}", bufs=1)
                nc.vector.tensor_copy(out=t, in_=xr[rc])
                xrb.append(t)
            xtb = []   # x^T bf16 [128, 385] x6
            for c in range(6):
                t = bp.tile([128, NV], BF16, tag=f"xtb{c}", name=f"xtb{c}")
                pst = ps_mm.tile([128, NV], BF16, tag="mm", name="pstx")
                for rc in range(4):
                    rows = 128 if rc < 3 else 1
                    nc.tensor.transpose(pst[:, rc * 128:rc * 128 + rows],
                                        xrb[rc][:, c * 128:(c + 1) * 128],
                                        identb if rc < 3 else identb[0:1, 0:1])
                nc.vector.tensor_copy(out=t, in_=pst)
                xtb.append(t)
            xe = []    # x^T fp32, x rows 64..384  -> [128, 321] x6
            for c in range(6):
                t = bp.tile([128, NQ], F32, tag=f"xe{c}", name=f"xe{c}", bufs=1)
                pst = ps_mm.tile([128, NQ], F32, tag="mm", name="pste")
                nc.tensor.transpose(pst[:, 0:64], xr[0][64:128, c * 128:(c + 1) * 128], identf[64:128, 64:128])
                nc.tensor.transpose(pst[:, 64:192], xr[1][:, c * 128:(c + 1) * 128], identf)
                nc.tensor.transpose(pst[:, 192:320], xr[2][:, c * 128:(c + 1) * 128], identf)
                nc.tensor.transpose(pst[:, 320:321], xr[3][0:1, c * 128:(c + 1) * 128], identf[0:1, 0:1])
                nc.vector.tensor_copy(out=t, in_=pst)
                xe.append(t)
            # xe cols: 0..63 = x rows 64..127; col j = x row 64+j
            # tgt mean over x rows 65..128 = xe cols 1..64
            tgt = []
            for c in range(6):
                t = ap_.tile([128, 1], F32, tag=f"tgt{c}", name=f"tgt{c}")
                nc.vector.tensor_reduce(out=t, in_=xe[c][:, 1:TPL + 1], axis=AX.X, op=ALU.add)
                nc.vector.tensor_scalar_mul(t, t, 1.0 / TPL)
                tgt.append(t)

            # --- divide MLP (fp32); xs^T = xe cols 65..320 (x rows 129..384) ---
            h1s = []
            for m in range(3):
                ps = ps_mm.tile([128, S], F32, tag="mm", name="psh1")
                for kc in range(6):
                    nc.tensor.matmul(ps, lhsT=dp1aT[kc][:, m * 128:(m + 1) * 128],
                                     rhs=xe[kc][:, 65:NQ],
                                     start=(kc == 0), stop=(kc == 5))
                pt = ps_sm.tile([128, 1], F32, tag="sm", name="pst1")
                for kc in range(6):
                    nc.tensor.matmul(pt, lhsT=dp1bT[kc][:, m * 128:(m + 1) * 128],
                                     rhs=tgt[kc], start=(kc == 0), stop=(kc == 5))
                t1 = ap_.tile([128, 1], F32, tag="t1", name="t1")
                nc.vector.tensor_scalar(out=t1, in0=pt, scalar1=dp1b_col[:, m:m + 1],
                                        scalar2=None, op0=ALU.add)
                h1 = bp.tile([128, S], F32, tag=f"h1_{m}", name=f"h1_{m}", bufs=1)
                nc.scalar.activation(out=h1, in_=ps, func=AF.Gelu, bias=t1, scale=1.0)
                h1s.append(h1)
            h2s = []
            for m in range(2):
                rows = 128 if m == 0 else 64
                c0, c1 = (0, 128) if m == 0 else (128, 192)
                ps = ps_mm.tile([128, S], F32, tag="mm", name="psh2")
                for kc in range(3):
                    nc.tensor.matmul(ps[0:rows], lhsT=dp2T[kc][:, c0:c1],
                                     rhs=h1s[kc], start=(kc == 0), stop=(kc == 2))
                h2 = bp.tile([128, S], F32, tag=f"h2_{m}", name=f"h2_{m}", bufs=1)
                bias_c = dp2b_c0 if m == 0 else dp2b_c1
                nc.scalar.activation(out=h2[0:rows], in_=ps[0:rows], func=AF.Gelu,
                                     bias=bias_c[0:rows], scale=1.0)
                h2s.append(h2)
            ldp = ps_sm.tile([1, S], F32, tag="sm", name="psld")
            nc.tensor.matmul(ldp, lhsT=w3d0, rhs=h2s[0], start=True, stop=False)
            nc.tensor.matmul(ldp, lhsT=w3d1, rhs=h2s[1][0:64], start=False, stop=True)

            # u row: 1 unless query is search-class1 (argmax==0, i.e. l1<=l0)
            u = ap_.tile([1, NQ], F32, tag="u", name="u")
            nc.vector.memset(u[0:1, 0:65], 1.0)
            nc.vector.tensor_scalar(out=u[0:1, 65:NQ], in0=ldp, scalar1=b3diff[0:1],
                                    scalar2=0.0, op0=ALU.add, op1=ALU.is_gt)
            kx = ap_.tile([1, NV], F32, tag="kx", name="kx")
            nc.vector.memset(kx[0:1, 0:129], 1.0)
            nc.vector.tensor_copy(out=kx[0:1, 129:NV], in_=u[0:1, 65:NQ])
            mcols = {}
            for kc in (1, 2):
                mp = ps_sm.tile([128, 1], F32, tag="sm", name="psmc")
                nc.tensor.transpose(mp, kx[0:1, kc * 128:(kc + 1) * 128], identf[0:1, 0:1])
                mc = ap_.tile([128, 1], F32, tag=f"mc{kc}", name=f"mc{kc}")
                nc.vector.tensor_copy(out=mc, in_=mp)
                mcols[kc] = mc
            m3 = ap_.tile([1, 1], F32, tag="m3", name="m3")
            nc.vector.tensor_copy(out=m3, in_=u[0:1, 320:321])
            ups = ps_att.tile([128, NQ], F32, tag="st", name="ups")
            nc.tensor.matmul(ups, lhsT=ones_row_f, rhs=u, start=True, stop=True)
            u_rep = ap_.tile([128, NQ], BF16, tag="urep", name="u_rep")
            nc.vector.tensor_copy(out=u_rep, in_=ups)
            nc.vector.memset(u_rep[0:1, :], 1.0)

            # --- q^T, k^T (bf16, feature-major) ---
            q_sb, k_sb = [], []
            for m in range(6):
                ps = ps_mm.tile([128, NV], F32, tag="mm", name="psq")
                for kc in range(6):
                    nc.tensor.matmul(ps, lhsT=wqkvT[kc][:, m * 128:(m + 1) * 128],
                                     rhs=xtb[kc], start=(kc == 0), stop=(kc == 5))
                q = bp.tile([128, NQ], BF16, tag=f"q{m}", name=f"q{m}")
                nc.scalar.activation(out=q[:, 0:1], in_=ps[:, 0:1], func=AF.Identity,
                                     bias=qb_s[:, m:m + 1], scale=SCALE)
                nc.scalar.activation(out=q[:, 1:NQ], in_=ps[:, 65:NV], func=AF.Identity,
                                     bias=qb_s[:, m:m + 1], scale=SCALE)
                q_sb.append(q)
            for m in range(6):
                ps = ps_mm.tile([128, NV], F32, tag="mm", name="psk")
                for kc in range(6):
                    nc.tensor.matmul(ps, lhsT=wqkvT[kc][:, (6 + m) * 128:(7 + m) * 128],
                                     rhs=xtb[kc], start=(kc == 0), stop=(kc == 5))
                k = bp.tile([128, NV], BF16, tag=f"k{m}", name=f"k{m}")
                nc.scalar.activation(out=k, in_=ps, func=AF.Identity,
                                     bias=qkvb_col[:, 6 + m:7 + m], scale=1.0)
                k_sb.append(k)

            # --- V (bf16, key-major, + ones column) ---
            v_sb = []
            for rc in range(3):
                vt = bp.tile([128, H, HD + 1], BF16, tag=f"v{rc}", name=f"v{rc}")
                for nh in range(2):
                    ps = ps_mm.tile([128, 384], F32, tag="mm", name="psv")
                    for kc in range(6):
                        nc.tensor.matmul(ps, lhsT=xtb[kc][:, rc * 128:(rc + 1) * 128],
                                         rhs=wqkvT[kc][:, 2 * C + nh * 384: 2 * C + (nh + 1) * 384],
                                         start=(kc == 0), stop=(kc == 5))
                    ps3 = ps[:, :].rearrange("p (h d) -> p h d", d=HD)
                    vbb = vb_rep[:, nh * 384:(nh + 1) * 384].rearrange("p (h d) -> p h d", d=HD)
                    nc.vector.tensor_tensor(out=vt[:, 6 * nh:6 * nh + 6, 0:HD], in0=ps3, in1=vbb, op=ALU.add)
                nc.vector.memset(vt[:, :, HD:HD + 1], 1.0)
                v_sb.append(vt)
            # vp3 row0 = pseudo eps-key (V=vsum, ones-slot=NQ), row1 = real key 384
            vp3 = bp.tile([2, H, HD + 1], BF16, tag="v3", name="vp3")
            tmpv = ap_.tile([1, H, HD + 1], BF16, tag="tmpv", name="tmpv")
            for nh in range(2):
                ps = ps_sm.tile([1, 384], F32, tag="sm", name="psv3")
                for kc in range(6):
                    nc.tensor.matmul(ps, lhsT=xtb[kc][:, 384:385],
                                     rhs=wqkvT[kc][:, 2 * C + nh * 384: 2 * C + (nh + 1) * 384],
                                     start=(kc == 0), stop=(kc == 5))
                ps3 = ps[:, :].rearrange("p (h d) -> p h d", d=HD)
                vbb = vb_rep[0:1, nh * 384:(nh + 1) * 384].rearrange("p (h d) -> p h d", d=HD)
                nc.vector.tensor_tensor(out=tmpv[0:1, 6 * nh:6 * nh + 6, 0:HD], in0=ps3, in1=vbb, op=ALU.add)
            nc.vector.memset(tmpv[0:1, :, HD:HD + 1], 1.0)
            for nh in range(2):
                vs_ps = ps_sm.tile([1, 384], F32, tag="sm", name="psvs")
                for rc in range(3):
                    nc.tensor.matmul(vs_ps, lhsT=ones_col,
                                     rhs=v_sb[rc][:, 6 * nh:6 * nh + 6, 0:HD],
                                     start=(rc == 0), stop=False)
                nc.tensor.matmul(vs_ps, lhsT=ones_col[0:1],
                                 rhs=tmpv[0:1, 6 * nh:6 * nh + 6, 0:HD],
                                 start=False, stop=True)
                nc.vector.tensor_copy(out=vp3[0:1, 6 * nh:6 * nh + 6, 0:HD],
                                      in_=vs_ps[0:1, :].rearrange("p (h d) -> p h d", d=HD))
            nc.vector.memset(vp3[0:1, :, HD:HD + 1], float(NQ))
            nc.sync.dma_start(out=vp3[1:2, :, :], in_=tmpv[0:1, :, :])

            # --- attention per head ---
            o_sb = [bp.tile([128, NQ], BF16, tag=f"o{cq}", name=f"o{cq}") for cq in range(6)]
            for h in range(H):
                cq, po = h // 2, 64 * (h % 2)
                at = []
                for kc in range(3):
                    sps = ps_att.tile([128, NQ], F32, tag="st", name="psst")
                    nc.tensor.matmul(sps, lhsT=k_sb[cq][po:po + 64, kc * 128:(kc + 1) * 128],
                                     rhs=q_sb[cq][po:po + 64, :], start=True, stop=True)
                    a = ap_.tile([128, NQ], BF16, tag=f"at{kc}", name=f"at{kc}")
                    nc.scalar.activation(out=a, in_=sps, func=AF.Exp)
                    at.append(a)
                sps3 = ps_sm.tile([1, NQ], F32, tag="sm", name="psst3")
                nc.tensor.matmul(sps3, lhsT=k_sb[cq][po:po + 64, 384:385],
                                 rhs=q_sb[cq][po:po + 64, :], start=True, stop=True)
                a3 = ap_.tile([2, NQ], BF16, tag="at3", name="at3")
                tmp3 = ap_.tile([1, NQ], BF16, tag="tmp3", name="tmp3")
                nc.scalar.activation(out=tmp3, in_=sps3, func=AF.Exp)
                nc.vector.tensor_scalar_mul(tmp3[0:1, 1:65], tmp3[0:1, 1:65], m3[0:1])
                nc.vector.memset(a3[0:1], EPS / NQ)
                nc.sync.dma_start(out=a3[1:2, :], in_=tmp3[0:1, :])
                # masks (u_rep row0 is all-ones so full-tile mult is safe for the token key)
                nc.vector.tensor_tensor(out=at[0], in0=at[0],
                                        in1=u_rep, op=ALU.mult)
                nc.vector.tensor_tensor(out=at[1][0:1, :], in0=at[1][0:1, :],
                                        in1=u[0:1, :], op=ALU.mult)
                nc.vector.tensor_scalar_mul(at[1][:, 1:65], at[1][:, 1:65], mcols[1])
                nc.vector.tensor_scalar_mul(at[2][:, 1:65], at[2][:, 1:65], mcols[2])
                # O^T (+ denominator via ones column)
                ops_ = ps_att.tile([65, NQ], F32, tag="ot", name="psot")
                for kc in range(3):
                    nc.tensor.matmul(ops_, lhsT=v_sb[kc][:, h:h + 1, :], rhs=at[kc],
                                     start=(kc == 0), stop=False)
                nc.tensor.matmul(ops_, lhsT=vp3[:, h:h + 1, :], rhs=a3, start=False, stop=True)
                r = ap_.tile([1, NQ], F32, tag="r", name="r")
                nc.vector.reciprocal(out=r, in_=ops_[64:65, :])
                rps = ps_att.tile([64, NQ], F32, tag="st", name="rps")
                nc.tensor.matmul(rps, lhsT=ones_row_f[0:1, 0:64], rhs=r, start=True, stop=True)
                r_rep = ap_.tile([64, NQ], F32, tag="rrep", name="r_rep")
                nc.vector.tensor_copy(out=r_rep, in_=rps)
                nc.vector.tensor_tensor(out=o_sb[cq][po:po + 64, :], in0=ops_[0:64, :],
                                        in1=r_rep, op=ALU.mult)

            # --- proj (row-major output) + contiguous store ---
            for qc in range(3):
                rows = 128 if qc < 2 else 65
                y = bp.tile([rows, C], F32, tag="y", name="y")
                for nh in range(2):
                    ps = ps_mm.tile([rows, 384], F32, tag="mm", name="psy")
                    for kc in range(6):
                        nc.tensor.matmul(ps, lhsT=o_sb[kc][:, qc * 128:qc * 128 + rows],
                                         rhs=wpT[kc][:, nh * 384:(nh + 1) * 384],
                                         start=(kc == 0), stop=(kc == 5))
                    nc.vector.tensor_tensor(out=y[:, nh * 384:(nh + 1) * 384], in0=ps,
                                            in1=projb_rep[0:rows, nh * 384:(nh + 1) * 384], op=ALU.add)
                nc.sync.dma_start(out=out_d[b, qc * 128:qc * 128 + rows, :], in_=y)
    return nc


def _make_nc(finalize=True):
    nc = bacc.Bacc(trn_type="TRN2")
    build(nc)
    if finalize:
        nc.finalize()
    return nc


def kernel(**inputs):
    from concourse.bass_utils import run_bass_kernel_spmd

    x = np.ascontiguousarray(np.asarray(inputs["x"], dtype=np.float32))
    B = x.shape[0]
    assert B == NCORES * BPC
    w_names = ["qkv_w", "qkv_b", "proj_w", "proj_b", "dp1_w", "dp1_b",
               "dp2_w", "dp2_b", "dp3_w", "dp3_b"]
    ws = {k: np.ascontiguousarray(np.asarray(inputs[k], dtype=np.float32)) for k in w_names}

    nc = _make_nc()
    in_maps = []
    for i in range(NCORES):
        m = {"x": x[i * BPC:(i + 1) * BPC]}
        m.update(ws)
        in_maps.append(m)
    res = run_bass_kernel_spmd(nc, in_maps, core_ids=list(range(NCORES)))
    out = np.concatenate([res.results[i]["out"] for i in range(NCORES)], axis=0)
    return out.astype(np.float32)


# revision 24
# speedup vs baseline: 1.1292x; 1.1292x over previous
"""Bass/Trainium2 kernel for nn_AttentionOU (sparse policy attention).

Contract: kernel(**inputs) takes FULL inputs (B=64), shards batch across 8
NeuronCores (8 per core), runs one SPMD Bass program, gathers full output.

Per-core program layout (all feature-major / key-major so matmuls compose
without runtime transposes of big intermediates):
  - x loaded row-major (contiguous DMA), cast to bf16, transposed on PE
    (128x128 tiles) into x^T bf16; rows 64..384 also transposed in fp32 for
    the divide-MLP (which must run fp32: bf16 flips ~32/16384 argmax
    decisions -> large output error).
  - q^T,k^T = Wq/Wk-stationary matmuls -> feature-major bf16.
  - V       = x^T-stationary matmuls -> key-major bf16 [128, 12, 65] tiles:
              per-head 64 V columns + a ones column (so the PV matmul also
              emits the softmax denominator for free).
  - S^T     = k^T.T @ q^T (key-major). No max-subtraction: |logits| <= ~3,
              exp is safe in fp32; eps-term difference is ~1e-8 absolute.
  - softmax: exp on ACT; the policy mask is two cheap row/col multiplies
              (mask only kills template<->search-class1 pairs); reference's
              (a + eps/Nq)/(sum + eps) reproduced exactly via a pseudo-key:
              a_pseudo = eps/Nq, V_pseudo = sum_j V_j, ones-slot = Nq.
  - O^T     = V.T @ a^T -> feature-major, feeds proj as stationary operand.
  - proj    = O^T-stationary -> y ROW-major -> contiguous stores.
"""

import numpy as np
from contextlib import ExitStack

import concourse.bass as bass
import concourse.tile as tile
from concourse import bacc, mybir
from concourse.masks import make_identity

F32 = mybir.dt.float32
BF16 = mybir.dt.bfloat16
AF = mybir.ActivationFunctionType
ALU = mybir.AluOpType
AX = mybir.AxisListType

BPC = 8          # batch per core
NCORES = 8
NV = 385         # keys
NQ = 321         # queries
C = 768
H = 12
HD = 64
S = 256
TPL = 64
EPS = 1e-6
SCALE = HD ** -0.5


def _bcast(t_ap, parts):
    """Broadcast a [1, N] AP across `parts` partitions (0-stride)."""
    return bass.AP(
        tensor=t_ap.tensor,
        offset=t_ap.offset,
        ap=[[0, parts]] + [list(d) for d in t_ap.ap[1:]],
    )


def _row_ap(dram_handle, off, n):
    return bass.AP(tensor=dram_handle[:].tensor, offset=off, ap=[[0, 1], [1, n]])


def build(nc):
    x_d = nc.dram_tensor("x", [BPC, NV, C], F32, kind="ExternalInput")
    qkvw_d = nc.dram_tensor("qkv_w", [3 * C, C], F32, kind="ExternalInput")
    qkvb_d = nc.dram_tensor("qkv_b", [3 * C], F32, kind="ExternalInput")
    projw_d = nc.dram_tensor("proj_w", [C, C], F32, kind="ExternalInput")
    projb_d = nc.dram_tensor("proj_b", [C], F32, kind="ExternalInput")
    dp1w_d = nc.dram_tensor("dp1_w", [384, 2 * C], F32, kind="ExternalInput")
    dp1b_d = nc.dram_tensor("dp1_b", [384], F32, kind="ExternalInput")
    dp2w_d = nc.dram_tensor("dp2_w", [192, 384], F32, kind="ExternalInput")
    dp2b_d = nc.dram_tensor("dp2_b", [192], F32, kind="ExternalInput")
    dp3w_d = nc.dram_tensor("dp3_w", [2, 192], F32, kind="ExternalInput")
    dp3b_d = nc.dram_tensor("dp3_b", [2], F32, kind="ExternalInput")
    out_d = nc.dram_tensor("out", [BPC, NQ, C], F32, kind="ExternalOutput")

    with tile.TileContext(nc) as tc, ExitStack() as ctx:
        wp = ctx.enter_context(tc.tile_pool(name="w", bufs=1))
        bp = ctx.enter_context(tc.tile_pool(name="bb", bufs=2))
        ap_ = ctx.enter_context(tc.tile_pool(name="aa", bufs=3))
        ps_mm = ctx.enter_context(tc.tile_pool(name="psmm", bufs=2, space="PSUM"))
        ps_att = ctx.enter_context(tc.tile_pool(name="psatt", bufs=4, space="PSUM"))
        ps_sm = ctx.enter_context(tc.tile_pool(name="pssm", bufs=2, space="PSUM"))

        identf = wp.tile([128, 128], F32, tag="identf")
        make_identity(nc, identf)
        identb = wp.tile([128, 128], BF16, tag="identb")
        nc.vector.tensor_copy(out=identb, in_=identf)
        ones_col = wp.tile([128, 1], BF16, tag="ones")
        nc.vector.memset(ones_col, 1.0)
        ones_row_f = wp.tile([1, 128], F32, tag="onesrf")
        nc.vector.memset(ones_row_f, 1.0)
        a3p_row = wp.tile([1, NQ], BF16, tag="a3p")
        nc.vector.memset(a3p_row, EPS / NQ)

        # ================= weights (once) =================
        stage_ctx = ExitStack()
        stage = stage_ctx.enter_context(tc.tile_pool(name="stage", bufs=1))
        # qkv_w -> bf16 row tiles -> transposed [infeat, 3C]
        wqkvT = [wp.tile([128, 3 * C], BF16, tag=f"wqkv{c}", name=f"wqkvT{c}") for c in range(6)]
        for r in range(18):
            t = stage.tile([128, C], BF16, tag="qkvrm", name=f"qkvrm{r}", bufs=1)
            nc.gpsimd.dma_start(out=t, in_=qkvw_d[r * 128:(r + 1) * 128, :])
            for c in range(6):
                pst = ps_mm.tile([128, 128], BF16, tag="mm", name="pst")
                nc.tensor.transpose(pst, t[:, c * 128:(c + 1) * 128], identb)
                nc.vector.tensor_copy(out=wqkvT[c][:, r * 128:(r + 1) * 128], in_=pst)
        wpT = [wp.tile([128, C], BF16, tag=f"wp{c}", name=f"wpT{c}") for c in range(6)]
        for r in range(6):
            t = stage.tile([128, C], BF16, tag="pjrm", name=f"pjrm{r}", bufs=2)
            nc.gpsimd.dma_start(out=t, in_=projw_d[r * 128:(r + 1) * 128, :])
            for c in range(6):
                pst = ps_mm.tile([128, 128], BF16, tag="mm", name="pst")
                nc.tensor.transpose(pst, t[:, c * 128:(c + 1) * 128], identb)
                nc.vector.tensor_copy(out=wpT[c][:, r * 128:(r + 1) * 128], in_=pst)
        # dp1_w fp32 [384, 1536] -> dp1aT/dp1bT [768, 384] chunks
        dp1aT = [wp.tile([128, 384], F32, tag=f"d1a{c}", name=f"d1aT{c}") for c in range(6)]
        dp1bT = [wp.tile([128, 384], F32, tag=f"d1b{c}", name=f"d1bT{c}") for c in range(6)]
        for r in range(3):
            t = stage.tile([128, 2 * C], F32, tag="d1rm", name=f"d1rm{r}", bufs=1)
            nc.sync.dma_start(out=t, in_=dp1w_d[r * 128:(r + 1) * 128, :])
            for c in range(6):
                for (dstl, base) in ((dp1aT, 0), (dp1bT, C)):
                    pst = ps_mm.tile([128, 128], F32, tag="mm", name="pst")
                    nc.tensor.transpose(pst, t[:, base + c * 128: base + (c + 1) * 128], identf)
                    nc.vector.tensor_copy(out=dstl[c][:, r * 128:(r + 1) * 128], in_=pst)
        dp2_rm0 = stage.tile([128, 384], F32, tag="d2rm0")
        nc.sync.dma_start(out=dp2_rm0, in_=dp2w_d[0:128, :])
        dp2_rm1 = stage.tile([64, 384], F32, tag="d2rm1")
        nc.sync.dma_start(out=dp2_rm1, in_=dp2w_d[128:192, :])
        dp2T = []
        for c in range(3):
            t = wp.tile([128, 192], F32, tag=f"d2{c}")
            pst = ps_mm.tile([128, 192], F32, tag="mm", name="pst")
            nc.tensor.transpose(pst[:, 0:128], dp2_rm0[:, c * 128:(c + 1) * 128], identf)
            nc.tensor.transpose(pst[:, 128:192], dp2_rm1[:, c * 128:(c + 1) * 128], identf[0:64, 0:64])
            nc.vector.tensor_copy(out=t, in_=pst)
            dp2T.append(t)
        dp3_rm = stage.tile([2, 192], F32, tag="d3rm")
        nc.sync.dma_start(out=dp3_rm, in_=dp3w_d[:, :])
        dp3T0 = wp.tile([128, 2], F32, tag="d3t0")
        pst = ps_sm.tile([128, 2], F32, tag="sm", name="pst3")
        nc.tensor.transpose(pst, dp3_rm[:, 0:128], identf[0:2, 0:2])
        nc.vector.tensor_copy(out=dp3T0, in_=pst)
        dp3T1 = wp.tile([64, 2], F32, tag="d3t1")
        pst = ps_sm.tile([64, 2], F32, tag="sm", name="pst4")
        nc.tensor.transpose(pst, dp3_rm[:, 128:192], identf[0:2, 0:2])
        nc.vector.tensor_copy(out=dp3T1, in_=pst)
        w3d0 = wp.tile([128, 1], F32, tag="w3d0")
        nc.vector.tensor_tensor(out=w3d0, in0=dp3T0[:, 1:2], in1=dp3T0[:, 0:1], op=ALU.subtract)
        w3d1 = wp.tile([64, 1], F32, tag="w3d1")
        nc.vector.tensor_tensor(out=w3d1, in0=dp3T1[:, 1:2], in1=dp3T1[:, 0:1], op=ALU.subtract)

        stage_ctx.close()

        # bias rows (contiguous) + transposed columns where needed
        qkvb_row = wp.tile([1, 3 * C], F32, tag="qkvbr")
        nc.sync.dma_start(out=qkvb_row, in_=_row_ap(qkvb_d, 0, 3 * C))
        qkvb_col = wp.tile([128, 12], F32, tag="qkvbc")
        for m in range(12):
            pst = ps_sm.tile([128, 1], F32, tag="sm", name="pstb")
            nc.tensor.transpose(pst, qkvb_row[0:1, m * 128:(m + 1) * 128], identf[0:1, 0:1])
            nc.vector.tensor_copy(out=qkvb_col[:, m:m + 1], in_=pst)
        qb_s = wp.tile([128, 6], F32, tag="qbs")
        nc.vector.tensor_scalar_mul(qb_s, qkvb_col[:, 0:6], SCALE)
        vb_rep = wp.tile([128, C], F32, tag="vbrep")
        nc.gpsimd.dma_start(out=vb_rep, in_=bass.AP(tensor=qkvb_d[:].tensor, offset=2 * C,
                                                    ap=[[0, 128], [1, C]]))
        projb_rep = wp.tile([128, C], F32, tag="pjbrep")
        nc.gpsimd.dma_start(out=projb_rep, in_=bass.AP(tensor=projb_d[:].tensor, offset=0,
                                                       ap=[[0, 128], [1, C]]))
        dp1b_row = wp.tile([1, 384], F32, tag="d1br")
        nc.sync.dma_start(out=dp1b_row, in_=_row_ap(dp1b_d, 0, 384))
        dp1b_col = wp.tile([128, 3], F32, tag="d1bc")
        for m in range(3):
            pst = ps_sm.tile([128, 1], F32, tag="sm", name="pstc")
            nc.tensor.transpose(pst, dp1b_row[0:1, m * 128:(m + 1) * 128], identf[0:1, 0:1])
            nc.vector.tensor_copy(out=dp1b_col[:, m:m + 1], in_=pst)
        dp2b_row = wp.tile([1, 192], F32, tag="d2br")
        nc.sync.dma_start(out=dp2b_row, in_=_row_ap(dp2b_d, 0, 192))
        dp2b_c0 = wp.tile([128, 1], F32, tag="d2b0")
        pst = ps_sm.tile([128, 1], F32, tag="sm", name="pstd")
        nc.tensor.transpose(pst, dp2b_row[0:1, 0:128], identf[0:1, 0:1])
        nc.vector.tensor_copy(out=dp2b_c0, in_=pst)
        dp2b_c1 = wp.tile([64, 1], F32, tag="d2b1")
        pst = ps_sm.tile([64, 1], F32, tag="sm", name="pste")
        nc.tensor.transpose(pst, dp2b_row[0:1, 128:192], identf[0:1, 0:1])
        nc.vector.tensor_copy(out=dp2b_c1, in_=pst)
        dp3b_row = wp.tile([1, 2], F32, tag="d3br")
        nc.sync.dma_start(out=dp3b_row, in_=_row_ap(dp3b_d, 0, 2))
        b3diff = wp.tile([1, 1], F32, tag="b3d")
        nc.vector.tensor_tensor(out=b3diff, in0=dp3b_row[0:1, 1:2], in1=dp3b_row[0:1, 0:1], op=ALU.subtract)

        # ================= per batch item =================
        for b in range(BPC):
            # --- load x row-major, cast, transpose ---
            xr = []
            for rc in range(4):
                rows = 128 if rc < 3 else 1
                t = bp.tile([rows, C], F32, tag=f"xr{rc}", name=f"xr{rc}", bufs=1)
                nc.sync.dma_start(out=t, in_=x_d[b, rc * 128:rc * 128 + rows, :])
                xr.append(t)
            xrb = []
            for rc in range(4):
                rows = 128 if rc < 3 else 1
                t = bp.tile([rows, C], BF16, tag=f"xrb{rc}", name=f"xrb{rc}", bufs=1)
                nc.gpsimd.dma_start(out=t, in_=x_d[b, rc * 128:rc * 128 + rows, :])
                xrb.append(t)
            xtb = []   # x^T bf16 [128, 385] x6
            for c in range(6):
                t = bp.tile([128, NV], BF16, tag=f"xtb{c}", name=f"xtb{c}")
                pst = ps_mm.tile([128, NV], BF16, tag="mm", name="pstx")
                for rc in range(4):
                    rows = 128 if rc < 3 else 1
                    nc.tensor.transpose(pst[:, rc * 128:rc * 128 + rows],
                                        xrb[rc][:, c * 128:(c + 1) * 128],
                                        identb if rc < 3 else identb[0:1, 0:1])
                nc.vector.tensor_copy(out=t, in_=pst)
                xtb.append(t)
            xe = []    # x^T fp32, x rows 64..384  -> [128, 321] x6
            for c in range(6):
                t = bp.tile([128, NQ], F32, tag=f"xe{c}", name=f"xe{c}", bufs=1)
                pst = ps_mm.tile([128, NQ], F32, tag="mm", name="pste")
                nc.tensor.transpose(pst[:, 0:64], xr[0][64:128, c * 128:(c + 1) * 128], identf[64:128, 64:128])
                nc.tensor.transpose(pst[:, 64:192], xr[1][:, c * 128:(c + 1) * 128], identf)
                nc.tensor.transpose(pst[:, 192:320], xr[2][:, c * 128:(c + 1) * 128], identf)
                nc.tensor.transpose(pst[:, 320:321], xr[3][0:1, c * 128:(c + 1) * 128], identf[0:1, 0:1])
                nc.vector.tensor_copy(out=t, in_=pst)
                xe.append(t)
            # xe cols: 0..63 = x rows 64..127; col j = x row 64+j
            # tgt mean over x rows 65..128 = xe cols 1..64
            tgt = []
            for c in range(6):
                t = ap_.tile([128, 1], F32, tag=f"tgt{c}", name=f"tgt{c}")
                nc.vector.tensor_reduce(out=t, in_=xe[c][:, 1:TPL + 1], axis=AX.X, op=ALU.add)
                nc.vector.tensor_scalar_mul(t, t, 1.0 / TPL)
                tgt.append(t)

            # --- divide MLP (fp32); xs^T = xe cols 65..320 (x rows 129..384) ---
            h1s = []
            for m in range(3):
                ps = ps_mm.tile([128, S], F32, tag="mm", name="psh1")
                for kc in range(6):
                    nc.tensor.matmul(ps, lhsT=dp1aT[kc][:, m * 128:(m + 1) * 128],
                                     rhs=xe[kc][:, 65:NQ],
                                     start=(kc == 0), stop=(kc == 5))
                pt = ps_sm.tile([128, 1], F32, tag="sm", name="pst1")
                for kc in range(6):
                    nc.tensor.matmul(pt, lhsT=dp1bT[kc][:, m * 128:(m + 1) * 128],
                                     rhs=tgt[kc], start=(kc == 0), stop=(kc == 5))
                t1 = ap_.tile([128, 1], F32, tag="t1", name="t1")
                nc.vector.tensor_scalar(out=t1, in0=pt, scalar1=dp1b_col[:, m:m + 1],
                                        scalar2=None, op0=ALU.add)
                h1 = bp.tile([128, S], F32, tag=f"h1_{m}", name=f"h1_{m}", bufs=1)
                nc.scalar.activation(out=h1, in_=ps, func=AF.Gelu, bias=t1, scale=1.0)
                h1s.append(h1)
            h2s = []
            for m in range(2):
                rows = 128 if m == 0 else 64
                c0, c1 = (0, 128) if m == 0 else (128, 192)
                ps = ps_mm.tile([128, S], F32, tag="mm", name="psh2")
                for kc in range(3):
                    nc.tensor.matmul(ps[0:rows], lhsT=dp2T[kc][:, c0:c1],
                                     rhs=h1s[kc], start=(kc == 0), stop=(kc == 2))
                h2 = bp.tile([128, S], F32, tag=f"h2_{m}", name=f"h2_{m}", bufs=1)
                bias_c = dp2b_c0 if m == 0 else dp2b_c1
                nc.scalar.activation(out=h2[0:rows], in_=ps[0:rows], func=AF.Gelu,
                                     bias=bias_c[0:rows], scale=1.0)
                h2s.append(h2)
            ldp = ps_sm.tile([1, S], F32, tag="sm", name="psld")
            nc.tensor.matmul(ldp, lhsT=w3d0, rhs=h2s[0], start=True, stop=False)
            nc.tensor.matmul(ldp, lhsT=w3d1, rhs=h2s[1][0:64], start=False, stop=True)

            # u row: 1 unless query is search-class1 (argmax==0, i.e. l1<=l0)
            u = ap_.tile([1, NQ], F32, tag="u", name="u")
            nc.vector.memset(u[0:1, 0:65], 1.0)
            nc.vector.tensor_scalar(out=u[0:1, 65:NQ], in0=ldp, scalar1=b3diff[0:1],
                                    scalar2=0.0, op0=ALU.add, op1=ALU.is_gt)
            kx = ap_.tile([1, NV], F32, tag="kx", name="kx")
            nc.vector.memset(kx[0:1, 0:129], 1.0)
            nc.vector.tensor_copy(out=kx[0:1, 129:NV], in_=u[0:1, 65:NQ])
            mcols = {}
            for kc in (1, 2):
                mp = ps_sm.tile([128, 1], F32, tag="sm", name="psmc")
                nc.tensor.transpose(mp, kx[0:1, kc * 128:(kc + 1) * 128], identf[0:1, 0:1])
                mc = ap_.tile([128, 1], F32, tag=f"mc{kc}", name=f"mc{kc}")
                nc.vector.tensor_copy(out=mc, in_=mp)
                mcols[kc] = mc
            m3 = ap_.tile([1, 1], F32, tag="m3", name="m3")
            nc.vector.tensor_copy(out=m3, in_=u[0:1, 320:321])
            ups = ps_att.tile([128, NQ], F32, tag="st", name="ups")
            nc.tensor.matmul(ups, lhsT=ones_row_f, rhs=u, start=True, stop=True)
            u_rep = ap_.tile([128, NQ], BF16, tag="urep", name="u_rep")
            nc.vector.tensor_copy(out=u_rep, in_=ups)
            nc.vector.memset(u_rep[0:1, :], 1.0)
            ub = ap_.tile([1, NQ], BF16, tag="ub", name="ub")
            nc.vector.tensor_copy(out=ub, in_=u)

            # --- q^T, k^T (bf16, feature-major) ---
            q_sb, k_sb = [], []
            for m in range(6):
                ps = ps_mm.tile([128, NV], F32, tag="mm", name="psq")
                for kc in range(6):
                    nc.tensor.matmul(ps, lhsT=wqkvT[kc][:, m * 128:(m + 1) * 128],
                                     rhs=xtb[kc], start=(kc == 0), stop=(kc == 5))
                q = bp.tile([128, NQ], BF16, tag=f"q{m}", name=f"q{m}")
                nc.scalar.activation(out=q[:, 0:1], in_=ps[:, 0:1], func=AF.Identity,
                                     bias=qb_s[:, m:m + 1], scale=SCALE)
                nc.scalar.activation(out=q[:, 1:NQ], in_=ps[:, 65:NV], func=AF.Identity,
                                     bias=qb_s[:, m:m + 1], scale=SCALE)
                q_sb.append(q)
            for m in range(6):
                ps = ps_mm.tile([128, NV], F32, tag="mm", name="psk")
                for kc in range(6):
                    nc.tensor.matmul(ps, lhsT=wqkvT[kc][:, (6 + m) * 128:(7 + m) * 128],
                                     rhs=xtb[kc], start=(kc == 0), stop=(kc == 5))
                k = bp.tile([128, NV], BF16, tag=f"k{m}", name=f"k{m}")
                nc.scalar.activation(out=k, in_=ps, func=AF.Identity,
                                     bias=qkvb_col[:, 6 + m:7 + m], scale=1.0)
                k_sb.append(k)

            # --- V (bf16, key-major, + ones column) ---
            v_sb = []
            for rc in range(3):
                vt = bp.tile([128, H, HD + 1], BF16, tag=f"v{rc}", name=f"v{rc}")
                for nh in range(2):
                    ps = ps_mm.tile([128, 384], F32, tag="mm", name="psv")
                    for kc in range(6):
                        nc.tensor.matmul(ps, lhsT=xtb[kc][:, rc * 128:(rc + 1) * 128],
                                         rhs=wqkvT[kc][:, 2 * C + nh * 384: 2 * C + (nh + 1) * 384],
                                         start=(kc == 0), stop=(kc == 5))
                    ps3 = ps[:, :].rearrange("p (h d) -> p h d", d=HD)
                    vbb = vb_rep[:, nh * 384:(nh + 1) * 384].rearrange("p (h d) -> p h d", d=HD)
                    nc.vector.tensor_tensor(out=vt[:, 6 * nh:6 * nh + 6, 0:HD], in0=ps3, in1=vbb, op=ALU.add)
                nc.vector.memset(vt[:, :, HD:HD + 1], 1.0)
                v_sb.append(vt)
            # real key 384 V-tile (tmpv) and pseudo eps-key V-tile (vp3p), both at partition 0
            tmpv = bp.tile([1, H, HD + 1], BF16, tag="tmpv", name="tmpv")
            vp3p = bp.tile([1, H, HD + 1], BF16, tag="v3p", name="vp3p")
            for nh in range(2):
                ps = ps_sm.tile([1, 384], F32, tag="sm", name="psv3")
                for kc in range(6):
                    nc.tensor.matmul(ps, lhsT=xtb[kc][:, 384:385],
                                     rhs=wqkvT[kc][:, 2 * C + nh * 384: 2 * C + (nh + 1) * 384],
                                     start=(kc == 0), stop=(kc == 5))
                ps3 = ps[:, :].rearrange("p (h d) -> p h d", d=HD)
                vbb = vb_rep[0:1, nh * 384:(nh + 1) * 384].rearrange("p (h d) -> p h d", d=HD)
                nc.vector.tensor_tensor(out=tmpv[0:1, 6 * nh:6 * nh + 6, 0:HD], in0=ps3, in1=vbb, op=ALU.add)
            nc.vector.memset(tmpv[0:1, :, HD:HD + 1], 1.0)
            for nh in range(2):
                vs_ps = ps_sm.tile([1, 384], F32, tag="sm", name="psvs")
                for rc in range(3):
                    nc.tensor.matmul(vs_ps, lhsT=ones_col,
                                     rhs=v_sb[rc][:, 6 * nh:6 * nh + 6, 0:HD],
                                     start=(rc == 0), stop=False)
                nc.tensor.matmul(vs_ps, lhsT=ones_col[0:1],
                                 rhs=tmpv[0:1, 6 * nh:6 * nh + 6, 0:HD],
                                 start=False, stop=True)
                nc.vector.tensor_copy(out=vp3p[0:1, 6 * nh:6 * nh + 6, 0:HD],
                                      in_=vs_ps[0:1, :].rearrange("p (h d) -> p h d", d=HD))
            nc.vector.memset(vp3p[0:1, :, HD:HD + 1], float(NQ))

            # --- attention per head ---
            o_sb = [bp.tile([128, NQ], BF16, tag=f"o{cq}", name=f"o{cq}") for cq in range(6)]
            for hp in range(6):
                heads = (2 * hp, 2 * hp + 1)
                at = {}
                tm3 = {}
                # scores: interleave the two heads (disjoint PE row groups -> concurrent)
                for kc in range(3):
                    for pi, h in enumerate(heads):
                        cq, po = h // 2, 64 * (h % 2)
                        sps = ps_att.tile([128, NQ], F32, tag="st", name="psst")
                        nc.tensor.matmul(sps, lhsT=k_sb[cq][po:po + 64, kc * 128:(kc + 1) * 128],
                                         rhs=q_sb[cq][po:po + 64, :], start=True, stop=True)
                        a = ap_.tile([128, NQ], BF16, tag=f"at{kc}p{pi}", name=f"at{kc}p{pi}", bufs=2)
                        nc.scalar.activation(out=a, in_=sps, func=AF.Exp)
                        at[(h, kc)] = a
                for pi, h in enumerate(heads):
                    cq, po = h // 2, 64 * (h % 2)
                    sps3 = ps_sm.tile([1, NQ], F32, tag="sm", name="psst3")
                    nc.tensor.matmul(sps3, lhsT=k_sb[cq][po:po + 64, 384:385],
                                     rhs=q_sb[cq][po:po + 64, :], start=True, stop=True)
                    t3 = ap_.tile([1, NQ], BF16, tag=f"tmp3p{pi}", name=f"tmp3p{pi}", bufs=2)
                    nc.scalar.activation(out=t3, in_=sps3, func=AF.Exp)
                    nc.vector.tensor_scalar_mul(t3[0:1, 1:65], t3[0:1, 1:65], m3[0:1])
                    tm3[h] = t3
                # masks
                for h in heads:
                    nc.vector.tensor_tensor(out=at[(h, 0)], in0=at[(h, 0)], in1=u_rep, op=ALU.mult)
                    nc.vector.tensor_tensor(out=at[(h, 1)][0:1, :], in0=at[(h, 1)][0:1, :],
                                            in1=ub[0:1, :], op=ALU.mult)
                    nc.vector.tensor_scalar_mul(at[(h, 1)][:, 1:65], at[(h, 1)][:, 1:65], mcols[1])
                    nc.vector.tensor_scalar_mul(at[(h, 2)][:, 1:65], at[(h, 2)][:, 1:65], mcols[2])
                # O^T (+ denominator via ones column) per head
                for h in heads:
                    cq, po = h // 2, 64 * (h % 2)
                    ops_ = ps_mm.tile([65, NQ], F32, tag="mm", name="psot")
                    for kc in range(3):
                        nc.tensor.matmul(ops_, lhsT=v_sb[kc][:, h:h + 1, :], rhs=at[(h, kc)],
                                         start=(kc == 0), stop=False)
                    nc.tensor.matmul(ops_, lhsT=tmpv[0:1, h:h + 1, :], rhs=tm3[h],
                                     start=False, stop=False)
                    nc.tensor.matmul(ops_, lhsT=vp3p[0:1, h:h + 1, :], rhs=a3p_row,
                                     start=False, stop=True)
                    r = ap_.tile([1, NQ], F32, tag="r", name="r")
                    nc.vector.reciprocal(out=r, in_=ops_[64:65, :])
                    rps = ps_att.tile([64, NQ], F32, tag="st", name="rps")
                    nc.tensor.matmul(rps, lhsT=ones_row_f[0:1, 0:64], rhs=r, start=True, stop=True)
                    r_rep = ap_.tile([64, NQ], F32, tag="rrep", name="r_rep")
                    nc.vector.tensor_copy(out=r_rep, in_=rps)
                    nc.vector.tensor_tensor(out=o_sb[cq][po:po + 64, :], in0=ops_[0:64, :],
                                            in1=r_rep, op=ALU.mult)

            # --- proj (row-major output) + contiguous store ---
            for qc in range(3):
                rows = 128 if qc < 2 else 65
                y = bp.tile([rows, C], F32, tag="y", name="y", bufs=1)
                for nh in range(2):
                    ps = ps_mm.tile([rows, 384], F32, tag="mm", name="psy")
                    for kc in range(6):
                        nc.tensor.matmul(ps, lhsT=o_sb[kc][:, qc * 128:qc * 128 + rows],
                                         rhs=wpT[kc][:, nh * 384:(nh + 1) * 384],
                                         start=(kc == 0), stop=(kc == 5))
                    nc.vector.tensor_tensor(out=y[:, nh * 384:(nh + 1) * 384], in0=ps,
                                            in1=projb_rep[0:rows, nh * 384:(nh + 1) * 384], op=ALU.add)
                nc.sync.dma_start(out=out_d[b, qc * 128:qc * 128 + rows, :], in_=y)
    return nc


def _make_nc(finalize=True):
    nc = bacc.Bacc(trn_type="TRN2")
    build(nc)
    if finalize:
        nc.finalize()
    return nc


def kernel(**inputs):
    from concourse.bass_utils import run_bass_kernel_spmd

    x = np.ascontiguousarray(np.asarray(inputs["x"], dtype=np.float32))
    B = x.shape[0]
    assert B == NCORES * BPC
    w_names = ["qkv_w", "qkv_b", "proj_w", "proj_b", "dp1_w", "dp1_b",
               "dp2_w", "dp2_b", "dp3_w", "dp3_b"]
    ws = {k: np.ascontiguousarray(np.asarray(inputs[k], dtype=np.float32)) for k in w_names}

    nc = _make_nc()
    in_maps = []
    for i in range(NCORES):
        m = {"x": x[i * BPC:(i + 1) * BPC]}
        m.update(ws)
        in_maps.append(m)
    res = run_bass_kernel_spmd(nc, in_maps, core_ids=list(range(NCORES)))
    out = np.concatenate([res.results[i]["out"] for i in range(NCORES)], axis=0)
    return out.astype(np.float32)


# revision 25
# speedup vs baseline: 1.2220x; 1.0822x over previous
"""Bass/Trainium2 kernel for nn_AttentionOU (sparse policy attention).

Contract: kernel(**inputs) takes FULL inputs (B=64), shards batch across 8
NeuronCores (8 per core), runs one SPMD Bass program, gathers full output.

Per-core program layout (all feature-major / key-major so matmuls compose
without runtime transposes of big intermediates):
  - x loaded row-major (contiguous DMA), cast to bf16, transposed on PE
    (128x128 tiles) into x^T bf16; rows 64..384 also transposed in fp32 for
    the divide-MLP (which must run fp32: bf16 flips ~32/16384 argmax
    decisions -> large output error).
  - q^T,k^T = Wq/Wk-stationary matmuls -> feature-major bf16.
  - V       = x^T-stationary matmuls -> key-major bf16 [128, 12, 65] tiles:
              per-head 64 V columns + a ones column (so the PV matmul also
              emits the softmax denominator for free).
  - S^T     = k^T.T @ q^T (key-major). No max-subtraction: |logits| <= ~3,
              exp is safe in fp32; eps-term difference is ~1e-8 absolute.
  - softmax: exp on ACT; the policy mask is two cheap row/col multiplies
              (mask only kills template<->search-class1 pairs); reference's
              (a + eps/Nq)/(sum + eps) reproduced exactly via a pseudo-key:
              a_pseudo = eps/Nq, V_pseudo = sum_j V_j, ones-slot = Nq.
  - O^T     = V.T @ a^T -> feature-major, feeds proj as stationary operand.
  - proj    = O^T-stationary -> y ROW-major -> contiguous stores.
"""

import numpy as np
from contextlib import ExitStack

import concourse.bass as bass
import concourse.tile as tile
from concourse import bacc, mybir
from concourse.masks import make_identity

F32 = mybir.dt.float32
BF16 = mybir.dt.bfloat16
AF = mybir.ActivationFunctionType
ALU = mybir.AluOpType
AX = mybir.AxisListType

BPC = 8          # batch per core
NCORES = 8
NV = 385         # keys
NQ = 321         # queries
C = 768
H = 12
HD = 64
S = 256
TPL = 64
EPS = 1e-6
SCALE = HD ** -0.5


def _bcast(t_ap, parts):
    """Broadcast a [1, N] AP across `parts` partitions (0-stride)."""
    return bass.AP(
        tensor=t_ap.tensor,
        offset=t_ap.offset,
        ap=[[0, parts]] + [list(d) for d in t_ap.ap[1:]],
    )


def _row_ap(dram_handle, off, n):
    return bass.AP(tensor=dram_handle[:].tensor, offset=off, ap=[[0, 1], [1, n]])


def build(nc):
    x_d = nc.dram_tensor("x", [BPC, NV, C], F32, kind="ExternalInput")
    qkvw_d = nc.dram_tensor("qkv_w", [3 * C, C], F32, kind="ExternalInput")
    qkvb_d = nc.dram_tensor("qkv_b", [3 * C], F32, kind="ExternalInput")
    projw_d = nc.dram_tensor("proj_w", [C, C], F32, kind="ExternalInput")
    projb_d = nc.dram_tensor("proj_b", [C], F32, kind="ExternalInput")
    dp1w_d = nc.dram_tensor("dp1_w", [384, 2 * C], F32, kind="ExternalInput")
    dp1b_d = nc.dram_tensor("dp1_b", [384], F32, kind="ExternalInput")
    dp2w_d = nc.dram_tensor("dp2_w", [192, 384], F32, kind="ExternalInput")
    dp2b_d = nc.dram_tensor("dp2_b", [192], F32, kind="ExternalInput")
    dp3w_d = nc.dram_tensor("dp3_w", [2, 192], F32, kind="ExternalInput")
    dp3b_d = nc.dram_tensor("dp3_b", [2], F32, kind="ExternalInput")
    out_d = nc.dram_tensor("out", [BPC, NQ, C], F32, kind="ExternalOutput")

    with tile.TileContext(nc) as tc, ExitStack() as ctx:
        wp = ctx.enter_context(tc.tile_pool(name="w", bufs=1))
        bp = ctx.enter_context(tc.tile_pool(name="bb", bufs=2))
        ap_ = ctx.enter_context(tc.tile_pool(name="aa", bufs=3))
        ps_mm = ctx.enter_context(tc.tile_pool(name="psmm", bufs=2, space="PSUM"))
        ps_att = ctx.enter_context(tc.tile_pool(name="psatt", bufs=4, space="PSUM"))
        ps_sm = ctx.enter_context(tc.tile_pool(name="pssm", bufs=2, space="PSUM"))

        identf = wp.tile([128, 128], F32, tag="identf")
        make_identity(nc, identf)
        identb = wp.tile([128, 128], BF16, tag="identb")
        nc.vector.tensor_copy(out=identb, in_=identf)
        ones_col = wp.tile([128, 1], BF16, tag="ones")
        nc.vector.memset(ones_col, 1.0)
        ones_row_f = wp.tile([1, 128], F32, tag="onesrf")
        nc.vector.memset(ones_row_f, 1.0)
        a3p_row = wp.tile([1, NQ], BF16, tag="a3p")
        nc.vector.memset(a3p_row, EPS / NQ)

        # ================= weights (once) =================
        stage_ctx = ExitStack()
        stage = stage_ctx.enter_context(tc.tile_pool(name="stage", bufs=1))
        # qkv_w -> bf16 row tiles -> transposed [infeat, 3C]
        wqkvT = [wp.tile([128, 3 * C], BF16, tag=f"wqkv{c}", name=f"wqkvT{c}") for c in range(6)]
        for r in range(18):
            t = stage.tile([128, C], BF16, tag="qkvrm", name=f"qkvrm{r}", bufs=1)
            nc.gpsimd.dma_start(out=t, in_=qkvw_d[r * 128:(r + 1) * 128, :])
            for c in range(6):
                pst = ps_mm.tile([128, 128], BF16, tag="mm", name="pst")
                nc.tensor.transpose(pst, t[:, c * 128:(c + 1) * 128], identb)
                nc.vector.tensor_copy(out=wqkvT[c][:, r * 128:(r + 1) * 128], in_=pst)
        wpT = [wp.tile([128, C], BF16, tag=f"wp{c}", name=f"wpT{c}") for c in range(6)]
        for r in range(6):
            t = stage.tile([128, C], BF16, tag="pjrm", name=f"pjrm{r}", bufs=2)
            nc.gpsimd.dma_start(out=t, in_=projw_d[r * 128:(r + 1) * 128, :])
            for c in range(6):
                pst = ps_mm.tile([128, 128], BF16, tag="mm", name="pst")
                nc.tensor.transpose(pst, t[:, c * 128:(c + 1) * 128], identb)
                nc.vector.tensor_copy(out=wpT[c][:, r * 128:(r + 1) * 128], in_=pst)
        # dp1_w fp32 [384, 1536] -> dp1aT/dp1bT [768, 384] chunks
        dp1aT = [wp.tile([128, 384], F32, tag=f"d1a{c}", name=f"d1aT{c}") for c in range(6)]
        dp1bT = [wp.tile([128, 384], F32, tag=f"d1b{c}", name=f"d1bT{c}") for c in range(6)]
        for r in range(3):
            t = stage.tile([128, 2 * C], F32, tag="d1rm", name=f"d1rm{r}", bufs=1)
            nc.sync.dma_start(out=t, in_=dp1w_d[r * 128:(r + 1) * 128, :])
            for c in range(6):
                for (dstl, base) in ((dp1aT, 0), (dp1bT, C)):
                    pst = ps_mm.tile([128, 128], F32, tag="mm", name="pst")
                    nc.tensor.transpose(pst, t[:, base + c * 128: base + (c + 1) * 128], identf)
                    nc.vector.tensor_copy(out=dstl[c][:, r * 128:(r + 1) * 128], in_=pst)
        dp2_rm0 = stage.tile([128, 384], F32, tag="d2rm0")
        nc.sync.dma_start(out=dp2_rm0, in_=dp2w_d[0:128, :])
        dp2_rm1 = stage.tile([64, 384], F32, tag="d2rm1")
        nc.sync.dma_start(out=dp2_rm1, in_=dp2w_d[128:192, :])
        dp2T = []
        for c in range(3):
            t = wp.tile([128, 192], F32, tag=f"d2{c}")
            pst = ps_mm.tile([128, 192], F32, tag="mm", name="pst")
            nc.tensor.transpose(pst[:, 0:128], dp2_rm0[:, c * 128:(c + 1) * 128], identf)
            nc.tensor.transpose(pst[:, 128:192], dp2_rm1[:, c * 128:(c + 1) * 128], identf[0:64, 0:64])
            nc.vector.tensor_copy(out=t, in_=pst)
            dp2T.append(t)
        dp3_rm = stage.tile([2, 192], F32, tag="d3rm")
        nc.sync.dma_start(out=dp3_rm, in_=dp3w_d[:, :])
        dp3T0 = wp.tile([128, 2], F32, tag="d3t0")
        pst = ps_sm.tile([128, 2], F32, tag="sm", name="pst3")
        nc.tensor.transpose(pst, dp3_rm[:, 0:128], identf[0:2, 0:2])
        nc.vector.tensor_copy(out=dp3T0, in_=pst)
        dp3T1 = wp.tile([64, 2], F32, tag="d3t1")
        pst = ps_sm.tile([64, 2], F32, tag="sm", name="pst4")
        nc.tensor.transpose(pst, dp3_rm[:, 128:192], identf[0:2, 0:2])
        nc.vector.tensor_copy(out=dp3T1, in_=pst)
        w3d0 = wp.tile([128, 1], F32, tag="w3d0")
        nc.vector.tensor_tensor(out=w3d0, in0=dp3T0[:, 1:2], in1=dp3T0[:, 0:1], op=ALU.subtract)
        w3d1 = wp.tile([64, 1], F32, tag="w3d1")
        nc.vector.tensor_tensor(out=w3d1, in0=dp3T1[:, 1:2], in1=dp3T1[:, 0:1], op=ALU.subtract)

        stage_ctx.close()

        # bias rows (contiguous) + transposed columns where needed
        qkvb_row = wp.tile([1, 3 * C], F32, tag="qkvbr")
        nc.sync.dma_start(out=qkvb_row, in_=_row_ap(qkvb_d, 0, 3 * C))
        qkvb_col = wp.tile([128, 12], F32, tag="qkvbc")
        for m in range(12):
            pst = ps_sm.tile([128, 1], F32, tag="sm", name="pstb")
            nc.tensor.transpose(pst, qkvb_row[0:1, m * 128:(m + 1) * 128], identf[0:1, 0:1])
            nc.vector.tensor_copy(out=qkvb_col[:, m:m + 1], in_=pst)
        qb_s = wp.tile([128, 6], F32, tag="qbs")
        nc.vector.tensor_scalar_mul(qb_s, qkvb_col[:, 0:6], SCALE)
        vb_rep = wp.tile([128, C], F32, tag="vbrep")
        nc.gpsimd.dma_start(out=vb_rep, in_=bass.AP(tensor=qkvb_d[:].tensor, offset=2 * C,
                                                    ap=[[0, 128], [1, C]]))
        projb_rep = wp.tile([128, C], F32, tag="pjbrep")
        nc.gpsimd.dma_start(out=projb_rep, in_=bass.AP(tensor=projb_d[:].tensor, offset=0,
                                                       ap=[[0, 128], [1, C]]))
        dp1b_row = wp.tile([1, 384], F32, tag="d1br")
        nc.sync.dma_start(out=dp1b_row, in_=_row_ap(dp1b_d, 0, 384))
        dp1b_col = wp.tile([128, 3], F32, tag="d1bc")
        for m in range(3):
            pst = ps_sm.tile([128, 1], F32, tag="sm", name="pstc")
            nc.tensor.transpose(pst, dp1b_row[0:1, m * 128:(m + 1) * 128], identf[0:1, 0:1])
            nc.vector.tensor_copy(out=dp1b_col[:, m:m + 1], in_=pst)
        dp2b_row = wp.tile([1, 192], F32, tag="d2br")
        nc.sync.dma_start(out=dp2b_row, in_=_row_ap(dp2b_d, 0, 192))
        dp2b_c0 = wp.tile([128, 1], F32, tag="d2b0")
        pst = ps_sm.tile([128, 1], F32, tag="sm", name="pstd")
        nc.tensor.transpose(pst, dp2b_row[0:1, 0:128], identf[0:1, 0:1])
        nc.vector.tensor_copy(out=dp2b_c0, in_=pst)
        dp2b_c1 = wp.tile([64, 1], F32, tag="d2b1")
        pst = ps_sm.tile([64, 1], F32, tag="sm", name="pste")
        nc.tensor.transpose(pst, dp2b_row[0:1, 128:192], identf[0:1, 0:1])
        nc.vector.tensor_copy(out=dp2b_c1, in_=pst)
        dp3b_row = wp.tile([1, 2], F32, tag="d3br")
        nc.sync.dma_start(out=dp3b_row, in_=_row_ap(dp3b_d, 0, 2))
        b3diff = wp.tile([1, 1], F32, tag="b3d")
        nc.vector.tensor_tensor(out=b3diff, in0=dp3b_row[0:1, 1:2], in1=dp3b_row[0:1, 0:1], op=ALU.subtract)

        # ================= per batch item =================
        for b in range(BPC):
            # --- load x row-major, cast, transpose ---
            xr = []
            for rc in range(4):
                rows = 128 if rc < 3 else 1
                t = bp.tile([rows, C], F32, tag=f"xr{rc}", name=f"xr{rc}", bufs=1)
                nc.sync.dma_start(out=t, in_=x_d[b, rc * 128:rc * 128 + rows, :])
                xr.append(t)
            xrb = []
            for rc in range(4):
                rows = 128 if rc < 3 else 1
                t = bp.tile([rows, C], BF16, tag=f"xrb{rc}", name=f"xrb{rc}", bufs=1)
                nc.gpsimd.dma_start(out=t, in_=x_d[b, rc * 128:rc * 128 + rows, :])
                xrb.append(t)
            xtb = []   # x^T bf16 [128, 385] x6
            for c in range(6):
                t = bp.tile([128, NV], BF16, tag=f"xtb{c}", name=f"xtb{c}")
                pst = ps_mm.tile([128, NV], BF16, tag="mm", name="pstx")
                for rc in range(4):
                    rows = 128 if rc < 3 else 1
                    nc.tensor.transpose(pst[:, rc * 128:rc * 128 + rows],
                                        xrb[rc][:, c * 128:(c + 1) * 128],
                                        identb if rc < 3 else identb[0:1, 0:1])
                nc.vector.tensor_copy(out=t, in_=pst)
                xtb.append(t)
            xe = []    # x^T fp32, x rows 64..384  -> [128, 321] x6
            for c in range(6):
                t = bp.tile([128, NQ], F32, tag=f"xe{c}", name=f"xe{c}", bufs=1)
                pst = ps_mm.tile([128, NQ], F32, tag="mm", name="pste")
                nc.tensor.transpose(pst[:, 0:64], xr[0][64:128, c * 128:(c + 1) * 128], identf[64:128, 64:128])
                nc.tensor.transpose(pst[:, 64:192], xr[1][:, c * 128:(c + 1) * 128], identf)
                nc.tensor.transpose(pst[:, 192:320], xr[2][:, c * 128:(c + 1) * 128], identf)
                nc.tensor.transpose(pst[:, 320:321], xr[3][0:1, c * 128:(c + 1) * 128], identf[0:1, 0:1])
                nc.vector.tensor_copy(out=t, in_=pst)
                xe.append(t)
            # xe cols: 0..63 = x rows 64..127; col j = x row 64+j
            # tgt mean over x rows 65..128 = xe cols 1..64
            tgt = []
            for c in range(6):
                t = ap_.tile([128, 1], F32, tag=f"tgt{c}", name=f"tgt{c}")
                nc.vector.tensor_reduce(out=t, in_=xe[c][:, 1:TPL + 1], axis=AX.X, op=ALU.add)
                nc.vector.tensor_scalar_mul(t, t, 1.0 / TPL)
                tgt.append(t)

            # --- divide MLP (fp32); xs^T = xe cols 65..320 (x rows 129..384) ---
            h1s = []
            for m in range(3):
                ps = ps_mm.tile([128, S], F32, tag="mm", name="psh1")
                for kc in range(6):
                    nc.tensor.matmul(ps, lhsT=dp1aT[kc][:, m * 128:(m + 1) * 128],
                                     rhs=xe[kc][:, 65:NQ],
                                     start=(kc == 0), stop=(kc == 5))
                pt = ps_sm.tile([128, 1], F32, tag="sm", name="pst1")
                for kc in range(6):
                    nc.tensor.matmul(pt, lhsT=dp1bT[kc][:, m * 128:(m + 1) * 128],
                                     rhs=tgt[kc], start=(kc == 0), stop=(kc == 5))
                t1 = ap_.tile([128, 1], F32, tag="t1", name="t1")
                nc.vector.tensor_scalar(out=t1, in0=pt, scalar1=dp1b_col[:, m:m + 1],
                                        scalar2=None, op0=ALU.add)
                h1 = bp.tile([128, S], F32, tag=f"h1_{m}", name=f"h1_{m}", bufs=1)
                nc.scalar.activation(out=h1, in_=ps, func=AF.Gelu, bias=t1, scale=1.0)
                h1s.append(h1)
            h2s = []
            for m in range(2):
                rows = 128 if m == 0 else 64
                c0, c1 = (0, 128) if m == 0 else (128, 192)
                ps = ps_mm.tile([128, S], F32, tag="mm", name="psh2")
                for kc in range(3):
                    nc.tensor.matmul(ps[0:rows], lhsT=dp2T[kc][:, c0:c1],
                                     rhs=h1s[kc], start=(kc == 0), stop=(kc == 2))
                h2 = bp.tile([128, S], F32, tag=f"h2_{m}", name=f"h2_{m}", bufs=1)
                bias_c = dp2b_c0 if m == 0 else dp2b_c1
                nc.scalar.activation(out=h2[0:rows], in_=ps[0:rows], func=AF.Gelu,
                                     bias=bias_c[0:rows], scale=1.0)
                h2s.append(h2)
            ldp = ps_sm.tile([1, S], F32, tag="sm", name="psld")
            nc.tensor.matmul(ldp, lhsT=w3d0, rhs=h2s[0], start=True, stop=False)
            nc.tensor.matmul(ldp, lhsT=w3d1, rhs=h2s[1][0:64], start=False, stop=True)

            # u row: 1 unless query is search-class1 (argmax==0, i.e. l1<=l0)
            u = ap_.tile([1, NQ], F32, tag="u", name="u")
            nc.vector.memset(u[0:1, 0:65], 1.0)
            nc.vector.tensor_scalar(out=u[0:1, 65:NQ], in0=ldp, scalar1=b3diff[0:1],
                                    scalar2=0.0, op0=ALU.add, op1=ALU.is_gt)
            kx = ap_.tile([1, NV], F32, tag="kx", name="kx")
            nc.vector.memset(kx[0:1, 0:129], 1.0)
            nc.vector.tensor_copy(out=kx[0:1, 129:NV], in_=u[0:1, 65:NQ])
            mcols = {}
            for kc in (1, 2):
                mp = ps_sm.tile([128, 1], F32, tag="sm", name="psmc")
                nc.tensor.transpose(mp, kx[0:1, kc * 128:(kc + 1) * 128], identf[0:1, 0:1])
                mc = ap_.tile([128, 1], F32, tag=f"mc{kc}", name=f"mc{kc}")
                nc.vector.tensor_copy(out=mc, in_=mp)
                mcols[kc] = mc
            m3 = ap_.tile([1, 1], F32, tag="m3", name="m3")
            nc.vector.tensor_copy(out=m3, in_=u[0:1, 320:321])
            ups = ps_att.tile([128, NQ], F32, tag="st", name="ups")
            nc.tensor.matmul(ups, lhsT=ones_row_f, rhs=u, start=True, stop=True)
            u_rep = ap_.tile([128, NQ], BF16, tag="urep", name="u_rep")
            nc.vector.tensor_copy(out=u_rep, in_=ups)
            nc.vector.memset(u_rep[0:1, :], 1.0)
            ub = ap_.tile([1, NQ], BF16, tag="ub", name="ub")
            nc.vector.tensor_copy(out=ub, in_=u)

            # --- q^T, k^T (bf16, feature-major) ---
            q_sb, k_sb = [], []
            for m in range(6):
                ps = ps_mm.tile([128, NV], F32, tag="mm", name="psq")
                for kc in range(6):
                    nc.tensor.matmul(ps, lhsT=wqkvT[kc][:, m * 128:(m + 1) * 128],
                                     rhs=xtb[kc], start=(kc == 0), stop=(kc == 5))
                q = bp.tile([128, NQ], BF16, tag=f"q{m}", name=f"q{m}")
                nc.scalar.activation(out=q[:, 0:1], in_=ps[:, 0:1], func=AF.Identity,
                                     bias=qb_s[:, m:m + 1], scale=SCALE)
                nc.scalar.activation(out=q[:, 1:NQ], in_=ps[:, 65:NV], func=AF.Identity,
                                     bias=qb_s[:, m:m + 1], scale=SCALE)
                q_sb.append(q)
            for m in range(6):
                ps = ps_mm.tile([128, NV], F32, tag="mm", name="psk")
                for kc in range(6):
                    nc.tensor.matmul(ps, lhsT=wqkvT[kc][:, (6 + m) * 128:(7 + m) * 128],
                                     rhs=xtb[kc], start=(kc == 0), stop=(kc == 5))
                k = bp.tile([128, NV], BF16, tag=f"k{m}", name=f"k{m}")
                nc.scalar.activation(out=k, in_=ps, func=AF.Identity,
                                     bias=qkvb_col[:, 6 + m:7 + m], scale=1.0)
                k_sb.append(k)

            # --- V (bf16, key-major, + ones column) ---
            v_sb = []
            for rc in range(3):
                vt = bp.tile([128, H, HD + 1], BF16, tag=f"v{rc}", name=f"v{rc}")
                for nh in range(2):
                    ps = ps_mm.tile([128, 384], F32, tag="mm", name="psv")
                    for kc in range(6):
                        nc.tensor.matmul(ps, lhsT=xtb[kc][:, rc * 128:(rc + 1) * 128],
                                         rhs=wqkvT[kc][:, 2 * C + nh * 384: 2 * C + (nh + 1) * 384],
                                         start=(kc == 0), stop=(kc == 5))
                    ps3 = ps[:, :].rearrange("p (h d) -> p h d", d=HD)
                    vbb = vb_rep[:, nh * 384:(nh + 1) * 384].rearrange("p (h d) -> p h d", d=HD)
                    nc.vector.tensor_tensor(out=vt[:, 6 * nh:6 * nh + 6, 0:HD], in0=ps3, in1=vbb, op=ALU.add)
                nc.vector.memset(vt[:, :, HD:HD + 1], 1.0)
                v_sb.append(vt)
            # real key 384 V-tile (tmpv) and pseudo eps-key V-tile (vp3p), both at partition 0
            tmpv = bp.tile([1, H, HD + 1], BF16, tag="tmpv", name="tmpv")
            vp3p = bp.tile([1, H, HD + 1], BF16, tag="v3p", name="vp3p")
            for nh in range(2):
                ps = ps_sm.tile([1, 384], F32, tag="sm", name="psv3")
                for kc in range(6):
                    nc.tensor.matmul(ps, lhsT=xtb[kc][:, 384:385],
                                     rhs=wqkvT[kc][:, 2 * C + nh * 384: 2 * C + (nh + 1) * 384],
                                     start=(kc == 0), stop=(kc == 5))
                ps3 = ps[:, :].rearrange("p (h d) -> p h d", d=HD)
                vbb = vb_rep[0:1, nh * 384:(nh + 1) * 384].rearrange("p (h d) -> p h d", d=HD)
                nc.vector.tensor_tensor(out=tmpv[0:1, 6 * nh:6 * nh + 6, 0:HD], in0=ps3, in1=vbb, op=ALU.add)
            nc.vector.memset(tmpv[0:1, :, HD:HD + 1], 1.0)
            for nh in range(2):
                vs_ps = ps_sm.tile([1, 384], F32, tag="sm", name="psvs")
                for rc in range(3):
                    nc.tensor.matmul(vs_ps, lhsT=ones_col,
                                     rhs=v_sb[rc][:, 6 * nh:6 * nh + 6, 0:HD],
                                     start=(rc == 0), stop=False)
                nc.tensor.matmul(vs_ps, lhsT=ones_col[0:1],
                                 rhs=tmpv[0:1, 6 * nh:6 * nh + 6, 0:HD],
                                 start=False, stop=True)
                nc.vector.tensor_copy(out=vp3p[0:1, 6 * nh:6 * nh + 6, 0:HD],
                                      in_=vs_ps[0:1, :].rearrange("p (h d) -> p h d", d=HD))
            nc.vector.memset(vp3p[0:1, :, HD:HD + 1], float(NQ))

            # --- attention per head ---
            o_sb = [bp.tile([128, NQ], BF16, tag=f"o{cq}", name=f"o{cq}") for cq in range(6)]
            for hp in range(6):
                heads = (2 * hp, 2 * hp + 1)
                at = {}
                tm3 = {}
                # scores: interleave the two heads (disjoint PE row groups -> concurrent)
                for kc in range(3):
                    for pi, h in enumerate(heads):
                        cq, po = h // 2, 64 * (h % 2)
                        sps = ps_att.tile([128, NQ], F32, tag="st", name="psst")
                        nc.tensor.matmul(sps, lhsT=k_sb[cq][po:po + 64, kc * 128:(kc + 1) * 128],
                                         rhs=q_sb[cq][po:po + 64, :], start=True, stop=True)
                        a = ap_.tile([128, NQ], BF16, tag=f"at{kc}p{pi}", name=f"at{kc}p{pi}", bufs=2)
                        nc.scalar.activation(out=a, in_=sps, func=AF.Exp)
                        at[(h, kc)] = a
                for pi, h in enumerate(heads):
                    cq, po = h // 2, 64 * (h % 2)
                    sps3 = ps_sm.tile([1, NQ], F32, tag="sm", name="psst3")
                    nc.tensor.matmul(sps3, lhsT=k_sb[cq][po:po + 64, 384:385],
                                     rhs=q_sb[cq][po:po + 64, :], start=True, stop=True)
                    t3 = ap_.tile([1, NQ], BF16, tag=f"tmp3p{pi}", name=f"tmp3p{pi}", bufs=2)
                    nc.scalar.activation(out=t3, in_=sps3, func=AF.Exp)
                    nc.vector.tensor_scalar_mul(t3[0:1, 1:65], t3[0:1, 1:65], m3[0:1])
                    tm3[h] = t3
                # masks
                for h in heads:
                    nc.vector.tensor_tensor(out=at[(h, 0)], in0=at[(h, 0)], in1=u_rep, op=ALU.mult)
                    nc.vector.tensor_tensor(out=at[(h, 1)][0:1, :], in0=at[(h, 1)][0:1, :],
                                            in1=ub[0:1, :], op=ALU.mult)
                    nc.vector.tensor_scalar_mul(at[(h, 1)][:, 1:65], at[(h, 1)][:, 1:65], mcols[1])
                    nc.vector.tensor_scalar_mul(at[(h, 2)][:, 1:65], at[(h, 2)][:, 1:65], mcols[2])
                # O^T (+ denominator via ones column) per head
                for h in heads:
                    cq, po = h // 2, 64 * (h % 2)
                    ops_ = ps_mm.tile([65, NQ], F32, tag="mm", name="psot")
                    for kc in range(3):
                        nc.tensor.matmul(ops_, lhsT=v_sb[kc][:, h:h + 1, :], rhs=at[(h, kc)],
                                         start=(kc == 0), stop=False)
                    nc.tensor.matmul(ops_, lhsT=tmpv[0:1, h:h + 1, :], rhs=tm3[h],
                                     start=False, stop=False)
                    nc.tensor.matmul(ops_, lhsT=vp3p[0:1, h:h + 1, :], rhs=a3p_row,
                                     start=False, stop=True)
                    cden = ap_.tile([1, NQ], F32, tag="cden", name="cden")
                    nc.vector.tensor_copy(out=cden, in_=ops_[64:65, :])
                    dps = ps_att.tile([64, NQ], F32, tag="st", name="dps")
                    nc.tensor.matmul(dps, lhsT=ones_row_f[0:1, 0:64], rhs=cden, start=True, stop=True)
                    r_rep = ap_.tile([64, NQ], F32, tag="rrep", name="r_rep")
                    nc.vector.reciprocal_approx_fast(out=r_rep, in_=dps)
                    nc.vector.tensor_tensor(out=o_sb[cq][po:po + 64, :], in0=ops_[0:64, :],
                                            in1=r_rep, op=ALU.mult)

            # --- proj (row-major output) + contiguous store ---
            for qc in range(3):
                rows = 128 if qc < 2 else 65
                y = bp.tile([rows, C], F32, tag="y", name="y", bufs=1)
                for nh in range(2):
                    ps = ps_mm.tile([rows, 384], F32, tag="mm", name="psy")
                    for kc in range(6):
                        nc.tensor.matmul(ps, lhsT=o_sb[kc][:, qc * 128:qc * 128 + rows],
                                         rhs=wpT[kc][:, nh * 384:(nh + 1) * 384],
                                         start=(kc == 0), stop=(kc == 5))
                    nc.vector.tensor_tensor(out=y[:, nh * 384:(nh + 1) * 384], in0=ps,
                                            in1=projb_rep[0:rows, nh * 384:(nh + 1) * 384], op=ALU.add)
                nc.sync.dma_start(out=out_d[b, qc * 128:qc * 128 + rows, :], in_=y)
    return nc


def _make_nc(finalize=True):
    nc = bacc.Bacc(trn_type="TRN2")
    build(nc)
    if finalize:
        nc.finalize()
    return nc


def kernel(**inputs):
    from concourse.bass_utils import run_bass_kernel_spmd

    x = np.ascontiguousarray(np.asarray(inputs["x"], dtype=np.float32))
    B = x.shape[0]
    assert B == NCORES * BPC
    w_names = ["qkv_w", "qkv_b", "proj_w", "proj_b", "dp1_w", "dp1_b",
               "dp2_w", "dp2_b", "dp3_w", "dp3_b"]
    ws = {k: np.ascontiguousarray(np.asarray(inputs[k], dtype=np.float32)) for k in w_names}

    nc = _make_nc()
    in_maps = []
    for i in range(NCORES):
        m = {"x": x[i * BPC:(i + 1) * BPC]}
        m.update(ws)
        in_maps.append(m)
    res = run_bass_kernel_spmd(nc, in_maps, core_ids=list(range(NCORES)))
    out = np.concatenate([res.results[i]["out"] for i in range(NCORES)], axis=0)
    return out.astype(np.float32)


# revision 26
# speedup vs baseline: 1.2790x; 1.0466x over previous
"""Bass/Trainium2 kernel for nn_AttentionOU (sparse policy attention).

Contract: kernel(**inputs) takes FULL inputs (B=64), shards batch across 8
NeuronCores (8 per core), runs one SPMD Bass program, gathers full output.

Per-core program layout (all feature-major / key-major so matmuls compose
without runtime transposes of big intermediates):
  - x loaded row-major (contiguous DMA), cast to bf16, transposed on PE
    (128x128 tiles) into x^T bf16; rows 64..384 also transposed in fp32 for
    the divide-MLP (which must run fp32: bf16 flips ~32/16384 argmax
    decisions -> large output error).
  - q^T,k^T = Wq/Wk-stationary matmuls -> feature-major bf16.
  - V       = x^T-stationary matmuls -> key-major bf16 [128, 12, 65] tiles:
              per-head 64 V columns + a ones column (so the PV matmul also
              emits the softmax denominator for free).
  - S^T     = k^T.T @ q^T (key-major). No max-subtraction: |logits| <= ~3,
              exp is safe in fp32; eps-term difference is ~1e-8 absolute.
  - softmax: exp on ACT; the policy mask is two cheap row/col multiplies
              (mask only kills template<->search-class1 pairs); reference's
              (a + eps/Nq)/(sum + eps) reproduced exactly via a pseudo-key:
              a_pseudo = eps/Nq, V_pseudo = sum_j V_j, ones-slot = Nq.
  - O^T     = V.T @ a^T -> feature-major, feeds proj as stationary operand.
  - proj    = O^T-stationary -> y ROW-major -> contiguous stores.
"""

import numpy as np
from contextlib import ExitStack

import concourse.bass as bass
import concourse.tile as tile
from concourse import bacc, mybir
from concourse.masks import make_identity

F32 = mybir.dt.float32
BF16 = mybir.dt.bfloat16
AF = mybir.ActivationFunctionType
ALU = mybir.AluOpType
AX = mybir.AxisListType

BPC = 8          # batch per core
NCORES = 8
NV = 385         # keys
NQ = 321         # queries
C = 768
H = 12
HD = 64
S = 256
TPL = 64
EPS = 1e-6
SCALE = HD ** -0.5


def _bcast(t_ap, parts):
    """Broadcast a [1, N] AP across `parts` partitions (0-stride)."""
    return bass.AP(
        tensor=t_ap.tensor,
        offset=t_ap.offset,
        ap=[[0, parts]] + [list(d) for d in t_ap.ap[1:]],
    )


def _row_ap(dram_handle, off, n):
    return bass.AP(tensor=dram_handle[:].tensor, offset=off, ap=[[0, 1], [1, n]])


def build(nc):
    x_d = nc.dram_tensor("x", [BPC, NV, C], F32, kind="ExternalInput")
    qkvw_d = nc.dram_tensor("qkv_w", [3 * C, C], F32, kind="ExternalInput")
    qkvb_d = nc.dram_tensor("qkv_b", [3 * C], F32, kind="ExternalInput")
    projw_d = nc.dram_tensor("proj_w", [C, C], F32, kind="ExternalInput")
    projb_d = nc.dram_tensor("proj_b", [C], F32, kind="ExternalInput")
    dp1w_d = nc.dram_tensor("dp1_w", [384, 2 * C], F32, kind="ExternalInput")
    dp1b_d = nc.dram_tensor("dp1_b", [384], F32, kind="ExternalInput")
    dp2w_d = nc.dram_tensor("dp2_w", [192, 384], F32, kind="ExternalInput")
    dp2b_d = nc.dram_tensor("dp2_b", [192], F32, kind="ExternalInput")
    dp3w_d = nc.dram_tensor("dp3_w", [2, 192], F32, kind="ExternalInput")
    dp3b_d = nc.dram_tensor("dp3_b", [2], F32, kind="ExternalInput")
    out_d = nc.dram_tensor("out", [BPC, NQ, C], F32, kind="ExternalOutput")

    with tile.TileContext(nc) as tc, ExitStack() as ctx:
        wp = ctx.enter_context(tc.tile_pool(name="w", bufs=1))
        bp = ctx.enter_context(tc.tile_pool(name="bb", bufs=2))
        ap_ = ctx.enter_context(tc.tile_pool(name="aa", bufs=3))
        ps_mm = ctx.enter_context(tc.tile_pool(name="psmm", bufs=2, space="PSUM"))
        ps_att = ctx.enter_context(tc.tile_pool(name="psatt", bufs=4, space="PSUM"))
        ps_sm = ctx.enter_context(tc.tile_pool(name="pssm", bufs=2, space="PSUM"))

        identf = wp.tile([128, 128], F32, tag="identf")
        make_identity(nc, identf)
        identb = wp.tile([128, 128], BF16, tag="identb")
        nc.vector.tensor_copy(out=identb, in_=identf)
        ones_col = wp.tile([128, 1], BF16, tag="ones")
        nc.vector.memset(ones_col, 1.0)
        ones_row_f = wp.tile([1, 128], F32, tag="onesrf")
        nc.vector.memset(ones_row_f, 1.0)
        a3p_row = wp.tile([1, NQ], BF16, tag="a3p")
        nc.vector.memset(a3p_row, EPS / NQ)

        # ================= weights (once) =================
        stage_ctx = ExitStack()
        stage = stage_ctx.enter_context(tc.tile_pool(name="stage", bufs=1))
        # qkv_w -> bf16 row tiles -> transposed [infeat, 3C]
        wqkvT = [wp.tile([128, 3 * C], BF16, tag=f"wqkv{c}", name=f"wqkvT{c}") for c in range(6)]
        for r in range(18):
            t = stage.tile([128, C], BF16, tag="qkvrm", name=f"qkvrm{r}", bufs=2)
            nc.gpsimd.dma_start(out=t, in_=qkvw_d[r * 128:(r + 1) * 128, :])
            for c in range(6):
                pst = ps_mm.tile([128, 128], BF16, tag="mm", name="pst")
                nc.tensor.transpose(pst, t[:, c * 128:(c + 1) * 128], identb)
                nc.vector.tensor_copy(out=wqkvT[c][:, r * 128:(r + 1) * 128], in_=pst)
        wpT = [wp.tile([128, C], BF16, tag=f"wp{c}", name=f"wpT{c}") for c in range(6)]
        for r in range(6):
            t = stage.tile([128, C], BF16, tag="pjrm", name=f"pjrm{r}", bufs=2)
            nc.gpsimd.dma_start(out=t, in_=projw_d[r * 128:(r + 1) * 128, :])
            for c in range(6):
                pst = ps_mm.tile([128, 128], BF16, tag="mm", name="pst")
                nc.tensor.transpose(pst, t[:, c * 128:(c + 1) * 128], identb)
                nc.vector.tensor_copy(out=wpT[c][:, r * 128:(r + 1) * 128], in_=pst)
        # dp1_w fp32 [384, 1536] -> dp1aT/dp1bT [768, 384] chunks
        dp1aT = [wp.tile([128, 384], F32, tag=f"d1a{c}", name=f"d1aT{c}") for c in range(6)]
        dp1bT = [wp.tile([128, 384], F32, tag=f"d1b{c}", name=f"d1bT{c}") for c in range(6)]
        for r in range(3):
            t = stage.tile([128, 2 * C], F32, tag="d1rm", name=f"d1rm{r}", bufs=1)
            nc.sync.dma_start(out=t, in_=dp1w_d[r * 128:(r + 1) * 128, :])
            for c in range(6):
                for (dstl, base) in ((dp1aT, 0), (dp1bT, C)):
                    pst = ps_mm.tile([128, 128], F32, tag="mm", name="pst")
                    nc.tensor.transpose(pst, t[:, base + c * 128: base + (c + 1) * 128], identf)
                    nc.vector.tensor_copy(out=dstl[c][:, r * 128:(r + 1) * 128], in_=pst)
        dp2_rm0 = stage.tile([128, 384], F32, tag="d2rm0")
        nc.sync.dma_start(out=dp2_rm0, in_=dp2w_d[0:128, :])
        dp2_rm1 = stage.tile([64, 384], F32, tag="d2rm1")
        nc.sync.dma_start(out=dp2_rm1, in_=dp2w_d[128:192, :])
        dp2T = []
        for c in range(3):
            t = wp.tile([128, 192], F32, tag=f"d2{c}")
            pst = ps_mm.tile([128, 192], F32, tag="mm", name="pst")
            nc.tensor.transpose(pst[:, 0:128], dp2_rm0[:, c * 128:(c + 1) * 128], identf)
            nc.tensor.transpose(pst[:, 128:192], dp2_rm1[:, c * 128:(c + 1) * 128], identf[0:64, 0:64])
            nc.vector.tensor_copy(out=t, in_=pst)
            dp2T.append(t)
        dp3_rm = stage.tile([2, 192], F32, tag="d3rm")
        nc.sync.dma_start(out=dp3_rm, in_=dp3w_d[:, :])
        dp3T0 = wp.tile([128, 2], F32, tag="d3t0")
        pst = ps_sm.tile([128, 2], F32, tag="sm", name="pst3")
        nc.tensor.transpose(pst, dp3_rm[:, 0:128], identf[0:2, 0:2])
        nc.vector.tensor_copy(out=dp3T0, in_=pst)
        dp3T1 = wp.tile([64, 2], F32, tag="d3t1")
        pst = ps_sm.tile([64, 2], F32, tag="sm", name="pst4")
        nc.tensor.transpose(pst, dp3_rm[:, 128:192], identf[0:2, 0:2])
        nc.vector.tensor_copy(out=dp3T1, in_=pst)
        w3d0 = wp.tile([128, 1], F32, tag="w3d0")
        nc.vector.tensor_tensor(out=w3d0, in0=dp3T0[:, 1:2], in1=dp3T0[:, 0:1], op=ALU.subtract)
        w3d1 = wp.tile([64, 1], F32, tag="w3d1")
        nc.vector.tensor_tensor(out=w3d1, in0=dp3T1[:, 1:2], in1=dp3T1[:, 0:1], op=ALU.subtract)

        stage_ctx.close()

        # bias rows (contiguous) + transposed columns where needed
        qkvb_row = wp.tile([1, 3 * C], F32, tag="qkvbr")
        nc.sync.dma_start(out=qkvb_row, in_=_row_ap(qkvb_d, 0, 3 * C))
        qkvb_col = wp.tile([128, 12], F32, tag="qkvbc")
        for m in range(12):
            pst = ps_sm.tile([128, 1], F32, tag="sm", name="pstb")
            nc.tensor.transpose(pst, qkvb_row[0:1, m * 128:(m + 1) * 128], identf[0:1, 0:1])
            nc.vector.tensor_copy(out=qkvb_col[:, m:m + 1], in_=pst)
        qb_s = wp.tile([128, 6], F32, tag="qbs")
        nc.vector.tensor_scalar_mul(qb_s, qkvb_col[:, 0:6], SCALE)
        vb_rep = wp.tile([128, C], F32, tag="vbrep")
        nc.gpsimd.dma_start(out=vb_rep, in_=bass.AP(tensor=qkvb_d[:].tensor, offset=2 * C,
                                                    ap=[[0, 128], [1, C]]))
        projb_rep = wp.tile([128, C], F32, tag="pjbrep")
        nc.gpsimd.dma_start(out=projb_rep, in_=bass.AP(tensor=projb_d[:].tensor, offset=0,
                                                       ap=[[0, 128], [1, C]]))
        dp1b_row = wp.tile([1, 384], F32, tag="d1br")
        nc.sync.dma_start(out=dp1b_row, in_=_row_ap(dp1b_d, 0, 384))
        dp1b_col = wp.tile([128, 3], F32, tag="d1bc")
        for m in range(3):
            pst = ps_sm.tile([128, 1], F32, tag="sm", name="pstc")
            nc.tensor.transpose(pst, dp1b_row[0:1, m * 128:(m + 1) * 128], identf[0:1, 0:1])
            nc.vector.tensor_copy(out=dp1b_col[:, m:m + 1], in_=pst)
        dp2b_row = wp.tile([1, 192], F32, tag="d2br")
        nc.sync.dma_start(out=dp2b_row, in_=_row_ap(dp2b_d, 0, 192))
        dp2b_c0 = wp.tile([128, 1], F32, tag="d2b0")
        pst = ps_sm.tile([128, 1], F32, tag="sm", name="pstd")
        nc.tensor.transpose(pst, dp2b_row[0:1, 0:128], identf[0:1, 0:1])
        nc.vector.tensor_copy(out=dp2b_c0, in_=pst)
        dp2b_c1 = wp.tile([64, 1], F32, tag="d2b1")
        pst = ps_sm.tile([64, 1], F32, tag="sm", name="pste")
        nc.tensor.transpose(pst, dp2b_row[0:1, 128:192], identf[0:1, 0:1])
        nc.vector.tensor_copy(out=dp2b_c1, in_=pst)
        dp3b_row = wp.tile([1, 2], F32, tag="d3br")
        nc.sync.dma_start(out=dp3b_row, in_=_row_ap(dp3b_d, 0, 2))
        b3diff = wp.tile([1, 1], F32, tag="b3d")
        nc.vector.tensor_tensor(out=b3diff, in0=dp3b_row[0:1, 1:2], in1=dp3b_row[0:1, 0:1], op=ALU.subtract)

        # ================= per batch item =================
        for b in range(BPC):
            # --- load x row-major, cast, transpose ---
            xr = []
            for rc in range(4):
                rows = 128 if rc < 3 else 1
                t = bp.tile([rows, C], F32, tag=f"xr{rc}", name=f"xr{rc}", bufs=1)
                nc.sync.dma_start(out=t, in_=x_d[b, rc * 128:rc * 128 + rows, :])
                xr.append(t)
            xrb = []
            for rc in range(4):
                rows = 128 if rc < 3 else 1
                t = bp.tile([rows, C], BF16, tag=f"xrb{rc}", name=f"xrb{rc}", bufs=1)
                nc.gpsimd.dma_start(out=t, in_=x_d[b, rc * 128:rc * 128 + rows, :])
                xrb.append(t)
            xtb = []   # x^T bf16 [128, 385] x6
            for c in range(6):
                t = bp.tile([128, NV], BF16, tag=f"xtb{c}", name=f"xtb{c}")
                pst = ps_mm.tile([128, NV], BF16, tag="mm", name="pstx")
                for rc in range(4):
                    rows = 128 if rc < 3 else 1
                    nc.tensor.transpose(pst[:, rc * 128:rc * 128 + rows],
                                        xrb[rc][:, c * 128:(c + 1) * 128],
                                        identb if rc < 3 else identb[0:1, 0:1])
                nc.vector.tensor_copy(out=t, in_=pst)
                xtb.append(t)
            xe = []    # x^T fp32, x rows 64..384  -> [128, 321] x6
            for c in range(6):
                t = bp.tile([128, NQ], F32, tag=f"xe{c}", name=f"xe{c}", bufs=1)
                pst = ps_mm.tile([128, NQ], F32, tag="mm", name="pste")
                nc.tensor.transpose(pst[:, 0:64], xr[0][64:128, c * 128:(c + 1) * 128], identf[64:128, 64:128])
                nc.tensor.transpose(pst[:, 64:192], xr[1][:, c * 128:(c + 1) * 128], identf)
                nc.tensor.transpose(pst[:, 192:320], xr[2][:, c * 128:(c + 1) * 128], identf)
                nc.tensor.transpose(pst[:, 320:321], xr[3][0:1, c * 128:(c + 1) * 128], identf[0:1, 0:1])
                nc.vector.tensor_copy(out=t, in_=pst)
                xe.append(t)
            # xe cols: 0..63 = x rows 64..127; col j = x row 64+j
            # tgt mean over x rows 65..128 = xe cols 1..64
            tgt = []
            for c in range(6):
                t = ap_.tile([128, 1], F32, tag=f"tgt{c}", name=f"tgt{c}")
                nc.vector.tensor_reduce(out=t, in_=xe[c][:, 1:TPL + 1], axis=AX.X, op=ALU.add)
                nc.vector.tensor_scalar_mul(t, t, 1.0 / TPL)
                tgt.append(t)

            # --- divide MLP (fp32); xs^T = xe cols 65..320 (x rows 129..384) ---
            h1s = []
            for m in range(3):
                ps = ps_mm.tile([128, S], F32, tag="mm", name="psh1")
                for kc in range(6):
                    nc.tensor.matmul(ps, lhsT=dp1aT[kc][:, m * 128:(m + 1) * 128],
                                     rhs=xe[kc][:, 65:NQ],
                                     start=(kc == 0), stop=(kc == 5))
                pt = ps_sm.tile([128, 1], F32, tag="sm", name="pst1")
                for kc in range(6):
                    nc.tensor.matmul(pt, lhsT=dp1bT[kc][:, m * 128:(m + 1) * 128],
                                     rhs=tgt[kc], start=(kc == 0), stop=(kc == 5))
                t1 = ap_.tile([128, 1], F32, tag="t1", name="t1")
                nc.vector.tensor_scalar(out=t1, in0=pt, scalar1=dp1b_col[:, m:m + 1],
                                        scalar2=None, op0=ALU.add)
                h1 = bp.tile([128, S], F32, tag=f"h1_{m}", name=f"h1_{m}", bufs=1)
                nc.scalar.activation(out=h1, in_=ps, func=AF.Gelu, bias=t1, scale=1.0)
                h1s.append(h1)
            h2s = []
            for m in range(2):
                rows = 128 if m == 0 else 64
                c0, c1 = (0, 128) if m == 0 else (128, 192)
                ps = ps_mm.tile([128, S], F32, tag="mm", name="psh2")
                for kc in range(3):
                    nc.tensor.matmul(ps[0:rows], lhsT=dp2T[kc][:, c0:c1],
                                     rhs=h1s[kc], start=(kc == 0), stop=(kc == 2))
                h2 = bp.tile([128, S], F32, tag=f"h2_{m}", name=f"h2_{m}", bufs=1)
                bias_c = dp2b_c0 if m == 0 else dp2b_c1
                nc.scalar.activation(out=h2[0:rows], in_=ps[0:rows], func=AF.Gelu,
                                     bias=bias_c[0:rows], scale=1.0)
                h2s.append(h2)
            ldp = ps_sm.tile([1, S], F32, tag="sm", name="psld")
            nc.tensor.matmul(ldp, lhsT=w3d0, rhs=h2s[0], start=True, stop=False)
            nc.tensor.matmul(ldp, lhsT=w3d1, rhs=h2s[1][0:64], start=False, stop=True)

            # u row: 1 unless query is search-class1 (argmax==0, i.e. l1<=l0)
            u = ap_.tile([1, NQ], F32, tag="u", name="u")
            nc.vector.memset(u[0:1, 0:65], 1.0)
            nc.vector.tensor_scalar(out=u[0:1, 65:NQ], in0=ldp, scalar1=b3diff[0:1],
                                    scalar2=0.0, op0=ALU.add, op1=ALU.is_gt)
            kx = ap_.tile([1, NV], F32, tag="kx", name="kx")
            nc.vector.memset(kx[0:1, 0:129], 1.0)
            nc.vector.tensor_copy(out=kx[0:1, 129:NV], in_=u[0:1, 65:NQ])
            mcols = {}
            for kc in (1, 2):
                mp = ps_sm.tile([128, 1], F32, tag="sm", name="psmc")
                nc.tensor.transpose(mp, kx[0:1, kc * 128:(kc + 1) * 128], identf[0:1, 0:1])
                mc = ap_.tile([128, 1], F32, tag=f"mc{kc}", name=f"mc{kc}")
                nc.vector.tensor_copy(out=mc, in_=mp)
                mcols[kc] = mc
            m3 = ap_.tile([1, 1], F32, tag="m3", name="m3")
            nc.vector.tensor_copy(out=m3, in_=u[0:1, 320:321])
            ups = ps_att.tile([128, NQ], F32, tag="st", name="ups")
            nc.tensor.matmul(ups, lhsT=ones_row_f, rhs=u, start=True, stop=True)
            u_rep = ap_.tile([128, NQ], BF16, tag="urep", name="u_rep")
            nc.vector.tensor_copy(out=u_rep, in_=ups)
            nc.vector.memset(u_rep[0:1, :], 1.0)
            ub = ap_.tile([1, NQ], BF16, tag="ub", name="ub")
            nc.vector.tensor_copy(out=ub, in_=u)

            # --- q^T, k^T (bf16, feature-major) ---
            q_sb, k_sb = [], []
            for m in range(6):
                ps = ps_mm.tile([128, NV], F32, tag="mm", name="psq")
                for kc in range(6):
                    nc.tensor.matmul(ps, lhsT=wqkvT[kc][:, m * 128:(m + 1) * 128],
                                     rhs=xtb[kc], start=(kc == 0), stop=(kc == 5))
                q = bp.tile([128, NQ], BF16, tag=f"q{m}", name=f"q{m}")
                nc.vector.tensor_scalar(out=q[:, 0:1], in0=ps[:, 0:1], scalar1=qkvb_col[:, m:m + 1],
                                        scalar2=SCALE, op0=ALU.add, op1=ALU.mult)
                nc.vector.tensor_scalar(out=q[:, 1:NQ], in0=ps[:, 65:NV], scalar1=qkvb_col[:, m:m + 1],
                                        scalar2=SCALE, op0=ALU.add, op1=ALU.mult)
                q_sb.append(q)
            for m in range(6):
                ps = ps_mm.tile([128, NV], F32, tag="mm", name="psk")
                for kc in range(6):
                    nc.tensor.matmul(ps, lhsT=wqkvT[kc][:, (6 + m) * 128:(7 + m) * 128],
                                     rhs=xtb[kc], start=(kc == 0), stop=(kc == 5))
                k = bp.tile([128, NV], BF16, tag=f"k{m}", name=f"k{m}")
                nc.vector.tensor_scalar(out=k, in0=ps, scalar1=qkvb_col[:, 6 + m:7 + m],
                                        scalar2=None, op0=ALU.add)
                k_sb.append(k)

            # --- V (bf16, key-major, + ones column) ---
            v_sb = []
            for rc in range(3):
                vt = bp.tile([128, H, HD + 1], BF16, tag=f"v{rc}", name=f"v{rc}")
                for nh in range(2):
                    ps = ps_mm.tile([128, 384], F32, tag="mm", name="psv")
                    for kc in range(6):
                        nc.tensor.matmul(ps, lhsT=xtb[kc][:, rc * 128:(rc + 1) * 128],
                                         rhs=wqkvT[kc][:, 2 * C + nh * 384: 2 * C + (nh + 1) * 384],
                                         start=(kc == 0), stop=(kc == 5))
                    ps3 = ps[:, :].rearrange("p (h d) -> p h d", d=HD)
                    vbb = vb_rep[:, nh * 384:(nh + 1) * 384].rearrange("p (h d) -> p h d", d=HD)
                    nc.vector.tensor_tensor(out=vt[:, 6 * nh:6 * nh + 6, 0:HD], in0=ps3, in1=vbb, op=ALU.add)
                nc.vector.memset(vt[:, :, HD:HD + 1], 1.0)
                v_sb.append(vt)
            # real key 384 V-tile (tmpv) and pseudo eps-key V-tile (vp3p), both at partition 0
            tmpv = bp.tile([1, H, HD + 1], BF16, tag="tmpv", name="tmpv", bufs=1)
            vp3p = bp.tile([1, H, HD + 1], BF16, tag="v3p", name="vp3p", bufs=1)
            for nh in range(2):
                ps = ps_sm.tile([1, 384], F32, tag="sm", name="psv3")
                for kc in range(6):
                    nc.tensor.matmul(ps, lhsT=xtb[kc][:, 384:385],
                                     rhs=wqkvT[kc][:, 2 * C + nh * 384: 2 * C + (nh + 1) * 384],
                                     start=(kc == 0), stop=(kc == 5))
                ps3 = ps[:, :].rearrange("p (h d) -> p h d", d=HD)
                vbb = vb_rep[0:1, nh * 384:(nh + 1) * 384].rearrange("p (h d) -> p h d", d=HD)
                nc.vector.tensor_tensor(out=tmpv[0:1, 6 * nh:6 * nh + 6, 0:HD], in0=ps3, in1=vbb, op=ALU.add)
            nc.vector.memset(tmpv[0:1, :, HD:HD + 1], 1.0)
            for nh in range(2):
                vs_ps = ps_sm.tile([1, 384], F32, tag="sm", name="psvs")
                for rc in range(3):
                    nc.tensor.matmul(vs_ps, lhsT=ones_col,
                                     rhs=v_sb[rc][:, 6 * nh:6 * nh + 6, 0:HD],
                                     start=(rc == 0), stop=False)
                nc.tensor.matmul(vs_ps, lhsT=ones_col[0:1],
                                 rhs=tmpv[0:1, 6 * nh:6 * nh + 6, 0:HD],
                                 start=False, stop=True)
                nc.vector.tensor_copy(out=vp3p[0:1, 6 * nh:6 * nh + 6, 0:HD],
                                      in_=vs_ps[0:1, :].rearrange("p (h d) -> p h d", d=HD))
            nc.vector.memset(vp3p[0:1, :, HD:HD + 1], float(NQ))

            # --- attention per head ---
            o_sb = [bp.tile([128, NQ], BF16, tag=f"o{cq}", name=f"o{cq}") for cq in range(6)]
            for hp in range(6):
                heads = (2 * hp, 2 * hp + 1)
                at = {}
                tm3 = {}
                # scores: interleave the two heads (disjoint PE row groups -> concurrent)
                for kc in range(3):
                    for pi, h in enumerate(heads):
                        cq, po = h // 2, 64 * (h % 2)
                        sps = ps_att.tile([128, NQ], F32, tag="st", name="psst")
                        nc.tensor.matmul(sps, lhsT=k_sb[cq][po:po + 64, kc * 128:(kc + 1) * 128],
                                         rhs=q_sb[cq][po:po + 64, :], start=True, stop=True)
                        a = ap_.tile([128, NQ], BF16, tag=f"at{kc}p{pi}", name=f"at{kc}p{pi}", bufs=2)
                        nc.scalar.activation(out=a, in_=sps, func=AF.Exp)
                        at[(h, kc)] = a
                for pi, h in enumerate(heads):
                    cq, po = h // 2, 64 * (h % 2)
                    sps3 = ps_sm.tile([1, NQ], F32, tag="sm", name="psst3")
                    nc.tensor.matmul(sps3, lhsT=k_sb[cq][po:po + 64, 384:385],
                                     rhs=q_sb[cq][po:po + 64, :], start=True, stop=True)
                    t3 = ap_.tile([1, NQ], BF16, tag=f"tmp3p{pi}", name=f"tmp3p{pi}", bufs=2)
                    nc.scalar.activation(out=t3, in_=sps3, func=AF.Exp)
                    nc.vector.tensor_scalar_mul(t3[0:1, 1:65], t3[0:1, 1:65], m3[0:1])
                    tm3[h] = t3
                # masks
                for h in heads:
                    nc.vector.tensor_tensor(out=at[(h, 0)], in0=at[(h, 0)], in1=u_rep, op=ALU.mult)
                    nc.vector.tensor_tensor(out=at[(h, 1)][0:1, :], in0=at[(h, 1)][0:1, :],
                                            in1=ub[0:1, :], op=ALU.mult)
                    nc.vector.tensor_scalar_mul(at[(h, 1)][:, 1:65], at[(h, 1)][:, 1:65], mcols[1])
                    nc.vector.tensor_scalar_mul(at[(h, 2)][:, 1:65], at[(h, 2)][:, 1:65], mcols[2])
                # O^T (+ denominator via ones column) per head
                for h in heads:
                    cq, po = h // 2, 64 * (h % 2)
                    ops_ = ps_mm.tile([65, NQ], F32, tag="mm", name="psot")
                    for kc in range(3):
                        nc.tensor.matmul(ops_, lhsT=v_sb[kc][:, h:h + 1, :], rhs=at[(h, kc)],
                                         start=(kc == 0), stop=False)
                    nc.tensor.matmul(ops_, lhsT=tmpv[0:1, h:h + 1, :], rhs=tm3[h],
                                     start=False, stop=False)
                    nc.tensor.matmul(ops_, lhsT=vp3p[0:1, h:h + 1, :], rhs=a3p_row,
                                     start=False, stop=True)
                    cden = ap_.tile([1, NQ], F32, tag="cden", name="cden")
                    nc.vector.tensor_copy(out=cden, in_=ops_[64:65, :])
                    dps = ps_att.tile([64, NQ], F32, tag="st", name="dps")
                    nc.tensor.matmul(dps, lhsT=ones_row_f[0:1, 0:64], rhs=cden, start=True, stop=True)
                    r_rep = ap_.tile([64, NQ], F32, tag="rrep", name="r_rep")
                    nc.vector.reciprocal_approx_fast(out=r_rep, in_=dps)
                    nc.vector.tensor_tensor(out=o_sb[cq][po:po + 64, :], in0=ops_[0:64, :],
                                            in1=r_rep, op=ALU.mult)

            # --- proj (row-major output) + contiguous store ---
            for qc in range(3):
                rows = 128 if qc < 2 else 65
                y = bp.tile([rows, C], F32, tag="y", name="y", bufs=1)
                for nh in range(2):
                    ps = ps_mm.tile([rows, 384], F32, tag="mm", name="psy")
                    for kc in range(6):
                        nc.tensor.matmul(ps, lhsT=o_sb[kc][:, qc * 128:qc * 128 + rows],
                                         rhs=wpT[kc][:, nh * 384:(nh + 1) * 384],
                                         start=(kc == 0), stop=(kc == 5))
                    nc.vector.tensor_tensor(out=y[:, nh * 384:(nh + 1) * 384], in0=ps,
                                            in1=projb_rep[0:rows, nh * 384:(nh + 1) * 384], op=ALU.add)
                nc.sync.dma_start(out=out_d[b, qc * 128:qc * 128 + rows, :], in_=y)
    return nc


def _make_nc(finalize=True):
    nc = bacc.Bacc(trn_type="TRN2")
    build(nc)
    if finalize:
        nc.finalize()
    return nc


def kernel(**inputs):
    from concourse.bass_utils import run_bass_kernel_spmd

    x = np.ascontiguousarray(np.asarray(inputs["x"], dtype=np.float32))
    B = x.shape[0]
    assert B == NCORES * BPC
    w_names = ["qkv_w", "qkv_b", "proj_w", "proj_b", "dp1_w", "dp1_b",
               "dp2_w", "dp2_b", "dp3_w", "dp3_b"]
    ws = {k: np.ascontiguousarray(np.asarray(inputs[k], dtype=np.float32)) for k in w_names}

    nc = _make_nc()
    in_maps = []
    for i in range(NCORES):
        m = {"x": x[i * BPC:(i + 1) * BPC]}
        m.update(ws)
        in_maps.append(m)
    res = run_bass_kernel_spmd(nc, in_maps, core_ids=list(range(NCORES)))
    out = np.concatenate([res.results[i]["out"] for i in range(NCORES)], axis=0)
    return out.astype(np.float32)
